# revision 24
# baseline (speedup 1.0000x reference)
"""Trainium2 Bass kernel for nn_AttentionBiLSTM_Seqence.

Model (S=512 seq, B=64 batch, E=512 emb, H=768 hidden, V=32000 vocab, L=32 labels):
  emb = embedding[tokens]                                  [S,B,E]
  forward LSTM scan (512 sequential steps, gate order r,f,g,o):
      g = xf_t + h @ Wh_f + bh_f ; c = sig(f)*c + sig(r)*tanh(g); h = sig(o)*tanh(c)
  backward direction: gb = emb@Wi_b + bi_b + hT@Wh_b + bh_b (hT = final fwd hidden),
      then a LINEAR reverse scan c2 = sig(f2)*c2 + sig(r2)*tanh(g2), hs_b = sig(o2)*tanh(c2)
  out = [hs_f, hs_b] @ Wout + bout ; out[:,:,0] += 10000*(tokens==1)

Sharding: batch-data-parallel over 8 cores (8 batch rows each), no collectives.

Per-core performance structure:
  * forward scan (16-step unrolled For_i, PE branch-prefetch hint): 4-way column-tiled
    matmuls (tile_position) stream Wh through the PE concurrently; gates live in a
    "gappy" [104,768] PSUM layout (quarter q of H at partitions 32q..32q+8) so
    sigmoid/tanh batch into 2 ACT instructions; emb@Wi_f is folded into the scan
    as 4 extra wave chunks read from the resident embT (staged per step into a
    fixed [128,32] tile by a full-width SBUF->SBUF DMA; bias enters via a K=1
    ones-row matmul that also inits PSUM) -- these waves are independent of
    h(t-1) and fill the PE idle during the sig(o)->h->transpose tail;
    h returns to lhsT layout via small PE transposes staged over 4 rotating
    PSUM banks (same-bank accumulation-group restarts stall the PE ~1.3us), chunk m
    of step u-1 interleaved ahead of gf wave k=m of step u; gf/rt gate tiles are
    single-buffered (their ACT reads land mid-step), only the o-gate tile is
    double-buffered (its sigmoid races the next step's injection).
  * backward scan: one tensor_tensor_scan per (h-chunk, batch) tile along reversed s.
  * all big matmuls bf16 with fp32 PSUM accumulation; recurrence state fp32.
"""

import sys
import numpy as np

sys.path.insert(0, "/opt/trn_rl_repo")

import ml_dtypes

import concourse.bass as bass
import concourse.bacc as bacc
import concourse.mybir as mybir
import concourse.tile as tile
from concourse.bass import ds, ts

BF16 = mybir.dt.bfloat16
F32 = mybir.dt.float32
I32 = mybir.dt.int32
AF = mybir.ActivationFunctionType
ALU = mybir.AluOpType

# -------- problem constants --------
S_FULL, B_FULL, E, H, V_FULL, L = 512, 64, 512, 768, 32000, 32
NCORES = 8
BC = B_FULL // NCORES          # batch per core = 8
PAD_WORD, PAD_BIAS = 1, 10000.0
G4 = 4 * H                     # 3072
QH = H // 4                    # 192 channels per quarter
U = 32                         # scan unroll per For_i iteration
UB = 16                        # hsfT block size (ring DMA'd out per half)
GP = 3 * 32 + BC               # 104: partitions covering the 4 gappy quarters

FULL_CFG = dict(S=S_FULL, V=V_FULL)


def fwd_perm():
    """perm[new] = old for the forward 4H axis: 4 quarters x [g|f|r|o] (192 each).

    g first so the whole tanh gate sits in PSUM bank A (cols 0:512) and its
    activation can overlap the bank-B (cols 512:768) matmul waves; f early so
    c *= sig(f) also overlaps them.
    """
    perm = np.zeros(G4, dtype=np.int64)
    base_of = {2: 0, 1: QH, 0: 2 * QH, 3: 3 * QH}  # orig gate idx (r,f,g,o) -> base
    for q in range(4):
        for g_orig, base in base_of.items():
            for j in range(QH):
                perm[q * 768 + base + j] = g_orig * H + q * QH + j
    return perm


def nchunks(total, maxn=512):
    out, o = [], 0
    while o < total:
        w = min(maxn, total - o)
        out.append((o, w))
        o += w
    return out


def load_cast_bf16(nc, tc, dst, src_dram, rows, cols):
    """DMA a [rows, cols] bf16 DRAM weight into dst bf16 SBUF [128, (rows/128)*cols]."""
    for k in range(rows // 128):
        nc.sync.dma_start(dst[:, k * cols:(k + 1) * cols],
                          src_dram[128 * k:128 * (k + 1), :])


def build_kernel(nc, tc, cfg):
    S, V = cfg["S"], cfg["V"]
    NBS = BC * S               # rows of (b,s), b-major
    NT = NBS // 128
    ST = S // 128              # 128-row s-tiles per batch row
    assert S % U == 0 and S % 128 == 0

    # ---------------- I/O ----------------
    tok_d = nc.dram_tensor("tokens_bm", [NBS], I32, kind="ExternalInput").ap()
    tokc_d = nc.dram_tensor("tokens_cm", [NBS], I32, kind="ExternalInput").ap()
    emb_d = nc.dram_tensor("embedding", [NBS, E], BF16, kind="ExternalInput").ap()
    wif_d = nc.dram_tensor("Wi_f_p", [E, G4], BF16, kind="ExternalInput").ap()
    whf_d = nc.dram_tensor("Wh_f_p", [H, G4], BF16, kind="ExternalInput").ap()
    bif_d = nc.dram_tensor("bi_f_p", [1, G4], F32, kind="ExternalInput").ap()
    bhf_d = nc.dram_tensor("bh_f_p", [1, G4], F32, kind="ExternalInput").ap()
    wib_d = nc.dram_tensor("Wi_b", [E, G4], BF16, kind="ExternalInput").ap()
    whb_d = nc.dram_tensor("Wh_b", [H, G4], BF16, kind="ExternalInput").ap()
    bib_d = nc.dram_tensor("bi_b", [1, G4], F32, kind="ExternalInput").ap()
    bhb_d = nc.dram_tensor("bh_b", [1, G4], F32, kind="ExternalInput").ap()
    wout_d = nc.dram_tensor("Wout", [2 * H, L], BF16, kind="ExternalInput").ap()
    bout_d = nc.dram_tensor("bout", [1, L], F32, kind="ExternalInput").ap()
    id128_d = nc.dram_tensor("id128", [128, 128], BF16, kind="ExternalInput").ap()
    id8_d = nc.dram_tensor("id8", [128, 8], BF16, kind="ExternalInput").ap()
    out_d = nc.dram_tensor("outT", [L, NBS], F32, kind="ExternalOutput").ap()

    from contextlib import ExitStack
    estack = ExitStack()
    glob = estack.enter_context(tc.tile_pool(name="glob", bufs=1))
    dram = estack.enter_context(tc.tile_pool(name="dram", bufs=1, space="DRAM"))

    embT = glob.tile([128, 4 * NBS], BF16)        # emb^T: [E-chunk k][(b,s) col]
    ring = glob.tile([128, 48 * U], BF16)         # h^T staging: col = m*(BC*U) + b*U + u
    id8 = glob.tile([128, 8], BF16)
    id128 = glob.tile([128, 128], BF16)
    qT = glob.tile([128, 24 * BC], F32)           # backward per-(chunk,b) bias columns

    whp = estack.enter_context(tc.tile_pool(name="whf", bufs=1))
    scst = estack.enter_context(tc.tile_pool(name="scst", bufs=1))
    xflp = estack.enter_context(tc.tile_pool(name="xfl", bufs=6))
    wh = whp.tile([128, 6 * G4], BF16)
    c_sb = scst.tile([128, QH], F32)
    sig = scst.tile([128, 3 * QH], BF16)
    tg = scst.tile([128, QH], BF16)
    tmp = scst.tile([128, QH], F32)
    tc_ = scst.tile([128, QH], BF16)
    h_sb = scst.tile([128, QH], BF16)
    scr = [scst.tile([128, 32], BF16, name=f"scr{j}") for j in range(2)]
    ring_v = ring.rearrange("p (m h b uu) -> p m h b uu", m=6, h=2, b=BC)

    hsfT_dram = dram.tile([6 * 128, NBS], BF16)   # row m*128+p = h-chan, col b*S+s
    hsbT_dram = dram.tile([6 * 128, NBS], BF16)

    nc.sync.dma_start(id8[:, :], id8_d)
    nc.sync.dma_start(id128[:, :], id128_d)

    # ---------------- phase 0: gather emb rows, transpose to embT ----------------
    with tc.tile_pool(name="gath", bufs=3) as gp, \
         tc.tile_pool(name="gathp", bufs=4, space="PSUM") as gpp, \
         tc.tile_pool(name="offs", bufs=1) as offp:
        tok_off = offp.tile([128, NT], I32)
        nc.sync.dma_start(tok_off[:, :], tokc_d.rearrange("(t p) -> p t", p=128))

        g_all = offp.tile([128, NT * E], BF16)
        for t in range(NT):
            nc.gpsimd.indirect_dma_start(
                out=g_all[:, t * E:(t + 1) * E], out_offset=None, in_=emb_d[:, :],
                in_offset=bass.IndirectOffsetOnAxis(ap=tok_off[:, t:t + 1], axis=0),
            )
        for t in range(NT):
            g_bf = g_all[:, t * E:(t + 1) * E]
            for e in range(4):
                pt = gpp.tile([128, 128], BF16)
                nc.tensor.transpose(pt[:, :], g_bf[:, 128 * e:128 * (e + 1)], id128[:, :])
                nc.scalar.activation(embT[:, e * NBS + t * 128: e * NBS + (t + 1) * 128],
                                     pt[:, :], AF.Copy)

    # ---------------- phase 2: forward LSTM scan ----------------
    # xf = emb @ Wi_f + bias is folded INTO the scan: per step, 4 extra wave
    # chunks contract embT (already resident, staged per-step into a fixed
    # [128, 32] tile via a full-width SBUF->SBUF DMA — the old [8,3072]
    # xf-row DMA wrote only 8 SBUF partitions, ~1/16 port width, ~1.5us/step)
    # and the bias enters via a K=1 ones-row matmul that also inits PSUM.
    # The emb waves don't depend on h(t-1), so they fill the PE idle while
    # the sig(o) -> h -> transpose chain of the previous step completes.
    with tc.tile_pool(name="scps", bufs=1, space="PSUM") as scps, \
         tc.tile_pool(name="wifp", bufs=1) as wifp:
        load_cast_bf16(nc, tc, wh, whf_d, H, G4)
        wif = wifp.tile([128, 4 * G4], BF16, name="wif")
        load_cast_bf16(nc, tc, wif, wif_d, E, G4)
        brow = whp.tile([1, G4], F32, name="brow")
        brow2 = whp.tile([1, G4], F32, name="brow2")
        nc.sync.dma_start(brow[0:1, :], bif_d)
        nc.sync.dma_start(brow2[0:1, :], bhf_d)
        nc.vector.tensor_add(brow[0:1, :], brow[0:1, :], brow2[0:1, :])
        biasf = whp.tile([1, G4], BF16, name="biasf")
        nc.vector.tensor_copy(biasf[:, :], brow[0:1, :])
        ones1 = whp.tile([1, BC], BF16, name="ones1")
        nc.vector.memset(ones1[:, :], 1.0)
        embT_v = embT.rearrange("p (j b s) -> p j b s", j=4, b=BC)
        # Separate PSUM tiles per gate group and parity: Tile serializes an
        # engine READ of a psum tile against PE WRITES to the same tile, so
        # each gate's activation can only overlap the later gates' matmul
        # waves if the gate groups live in different tiles.
        # Layout per quadrant (perm [g|f|r|o]): gf = cols 0:384, rt = 384:576,
        # ot = 576:768.
        # gf/rt single-buffered: their activations fire right after their own
        # wave groups (early in the step), long before the next step's
        # injection; only the o-gate tile needs parity (sig(o) is the step
        # tail, racing the next step's injection). The freed PSUM banks give
        # the h^T transpose staging a 4-deep rotation — consecutive
        # accumulation-group restarts on a just-drained PSUM bank stall the
        # PE ~1.3us (HW-measured), so spreading the per-step transposes over
        # 4 banks instead of 2 doubles the reuse gap.
        pgf = scps.tile([128, 384], F32, tag="pgf", name="pgf")
        prt = scps.tile([128, 192], F32, tag="prt", name="prt")
        pot = [scps.tile([128, 192], F32, tag=f"pot{j}", name=f"pot{j}") for j in range(2)]
        pts = [scps.tile([128, 24], BF16, tag=f"pt{j}", name=f"pt{j}") for j in range(4)]

        nc.vector.memset(c_sb[:, :], 0.0)
        nc.vector.memset(h_sb[:, :], 0.0)
        nc.vector.memset(ring[:, :], 0.0)
        nc.vector.memset(pgf[:, :], 0.0)
        nc.vector.memset(prt[:, :], 0.0)
        for j in range(2):
            nc.vector.memset(pot[j][:, :], 0.0)

        # transposes chunk-major ([8,128] single-shot where the chunk
        # lies in one quarter); staging tiles rotate over 4 PSUM banks so a
        # bank's next accumulation-group restart is 4 chunk-slots away
        CHUNK_PIECES = (
            ((0, 0, 128, 0, 128),),                    # m=0: q0 cols 0:128
            ((0, 128, 64, 0, 64), (1, 0, 64, 64, 64)),  # m=1
            ((1, 64, 128, 0, 128),),                   # m=2
            ((2, 0, 128, 0, 128),),                    # m=3
            ((2, 128, 64, 0, 64), (3, 0, 64, 64, 64)),  # m=4
            ((3, 64, 128, 0, 128),),                   # m=5
        )

        # chunks 3-5 transpose on the (otherwise idle) DVE via 32x32
        # stream-transpose blocks into a scratch tile (junk cols 8:32 come
        # from the zeroed gap rows of h_sb and are ignored by the 8-col
        # copy); chunks 0-2 stay on PE. Splitting the transposes across
        # engines shortens the PE's post-h serial block (HW: -0.37us/step).
        DVE_BLOCKS = {
            3: ((2, 0), (2, 32), (2, 64), (2, 96)),
            4: ((2, 128), (2, 160), (3, 0), (3, 32)),
            5: ((3, 64), (3, 96), (3, 128), (3, 160)),
        }

        def emit_transpose_chunk(u, m):
            if m >= 3:
                sc = scr[m % 2]
                for j, (q, c0) in enumerate(DVE_BLOCKS[m]):
                    nc.vector.transpose(
                        sc[32 * j:32 * (j + 1), 0:32],
                        h_sb[32 * q:32 * q + 32, c0:c0 + 32])
                nc.vector.tensor_copy(ring_v[:, m, u // UB, :, u % UB],
                                      sc[:, 0:BC])
                return
            ptm = pts[(6 * u + m) % 4]
            for (q, c0, cw, p0, pw) in CHUNK_PIECES[m]:
                nc.tensor.matmul(
                    ptm[p0:p0 + pw, 0:BC],
                    h_sb[32 * q:32 * q + BC, c0:c0 + cw],
                    id8[32 * q:32 * q + BC, :BC],
                    is_transpose=True, tile_position=(32 * q, p0),
                    skip_group_check=True)
            nc.vector.tensor_copy(ring_v[:, m, u // UB, :, u % UB],
                                  ptm[:, 0:BC])

        def emit_transposes(u):
            for m in range(6):
                emit_transpose_chunk(u, m)

        # hsfT_dram column layout: col = (s//U)*(BC*U) + b*U + (s%U)
        with tc.For_i(0, S, U, hint_engines=(mybir.EngineType.PE, mybir.EngineType.DVE)) as i:
            for u in range(U):
                ggf = pgf
                grt = prt
                got = pot[u % 2]
                prev = (u - 1) % U
                # stage embT columns for this step (and prefetch next):
                # xe[:, j*8+b] = embT[:, j*NBS + b*S + (i+u)]
                if u == 0:
                    xe0 = xflp.tile([128, 4 * BC], BF16, tag="xe", name="xe0")
                    nc.sync.dma_start(xe0[:, :], embT_v[:, :, :, ds(i, 1)])
                    xes = {0: xe0}
                if u < U - 1:
                    xen = xflp.tile([128, 4 * BC], BF16, tag="xe", name=f"xe{u + 1}")
                    (nc.gpsimd if u % 2 == 0 else nc.scalar).dma_start(
                        xen[:, :], embT_v[:, :, :, ds(i + u + 1, 1)])
                    xes[u + 1] = xen
                xe = xes.pop(u)
                # bias K=1 injection (inits PSUM), then the 4 emb wave chunks;
                # neither depends on h(u-1)
                for dst, c0, cw in ((ggf, 0, 384), (grt, 384, 192), (got, 576, 192)):
                    for q in range(4):
                        nc.tensor.matmul(
                            dst[32 * q:32 * q + BC, :],
                            lhsT=ones1[0:1, 0:BC],
                            rhs=biasf[0:1, q * 768 + c0: q * 768 + c0 + cw],
                            start=True, stop=False, tile_position=(0, 32 * q),
                            skip_group_check=True)
                for dst, c0, cw in ((ggf, 0, 384), (grt, 384, 192), (got, 576, 192)):
                    for j in range(4):
                        for q in range(4):
                            nc.tensor.matmul(
                                dst[32 * q:32 * q + BC, :],
                                lhsT=xe[:, j * BC:(j + 1) * BC],
                                rhs=wif[:, j * G4 + q * 768 + c0: j * G4 + q * 768 + c0 + cw],
                                start=False, stop=False, tile_position=(0, 32 * q),
                                skip_group_check=True)
                # k-waves gate-group major: g+f first, then r, then o. Each
                # group's activation fires as soon as its own tile's waves end
                # and overlaps the later groups' waves; only sigmoid(o) and
                # h = sig(o)*tanh(c) remain exposed after the last wave.
                # Step u-1's transpose chunk m is interleaved just before the
                # gf wave k=m of step u: only chunk 0's transpose+ring-copy is
                # chain-exposed after h(u-1); chunks 1-5 hide inside the wave
                # stream (wave k only needs ring chunk k).
                for dst, c0, cw in ((ggf, 0, 384), (grt, 384, 192), (got, 576, 192)):
                    for k in range(6):
                        if dst is ggf and u > 0:
                            emit_transpose_chunk(u - 1, k)
                        for q in range(4):
                            nc.tensor.matmul(
                                dst[32 * q:32 * q + BC, :],
                                lhsT=ring_v[:, k, prev // UB, :, prev % UB],
                                rhs=wh[:, k * G4 + q * 768 + c0: k * G4 + q * 768 + c0 + cw],
                                start=False, stop=(k == 5), tile_position=(0, 32 * q),
                                skip_group_check=True)
                    if dst is ggf:
                        if u == UB:
                            # first half of the ring is complete: stream it
                            # out now so the DMAs overlap the second half
                            for m in range(6):
                                nc.gpsimd.dma_start(
                                    hsfT_dram[m * 128:(m + 1) * 128, ds(i * BC, 128)],
                                    ring[:, m * (BC * U): m * (BC * U) + 128])
                        nc.scalar.activation(tg[0:GP, :], ggf[0:GP, 0:QH], AF.Tanh)
                        nc.scalar.activation(sig[0:GP, 0:QH], ggf[0:GP, QH:2 * QH],
                                             AF.Sigmoid)
                    elif dst is grt:
                        nc.scalar.activation(sig[0:GP, QH:2 * QH], grt[0:GP, :],
                                             AF.Sigmoid)
                nc.scalar.activation(sig[0:GP, 2 * QH:3 * QH], got[0:GP, :], AF.Sigmoid)
                # sig semantic layout: [f 0:192 | r 192:384 | o 384:576]
                nc.vector.tensor_mul(c_sb[0:GP, :], c_sb[0:GP, :], sig[0:GP, 0:QH])
                nc.vector.tensor_mul(tmp[0:GP, :], sig[0:GP, QH:2 * QH], tg[0:GP, :])
                nc.vector.tensor_add(c_sb[0:GP, :], c_sb[0:GP, :], tmp[0:GP, :])
                nc.scalar.activation(tc_[0:GP, :], c_sb[0:GP, :], AF.Tanh)
                nc.vector.tensor_mul(h_sb[0:GP, :], sig[0:GP, 2 * QH:3 * QH], tc_[0:GP, :])
            emit_transposes(U - 1)
            for m in range(6):
                nc.gpsimd.dma_start(
                    hsfT_dram[m * 128:(m + 1) * 128, ds(i * BC + 128, 128)],
                    ring[:, m * (BC * U) + 128: m * (BC * U) + 256])

    # ---------------- phase 3: qT = Wh_b^T @ hT + (bi_b + bh_b) ----------------
    with tc.tile_pool(name="whb", bufs=1) as qwp, \
         tc.tile_pool(name="qps", bufs=4, space="PSUM") as qpp, \
         tc.tile_pool(name="qtmp", bufs=1) as qtp:
        whb = qwp.tile([128, 6 * G4], BF16)
        load_cast_bf16(nc, tc, whb, whb_d, H, G4)
        bb = qtp.tile([1, G4], F32, tag="bb")
        bb2 = qtp.tile([1, G4], F32, tag="bb2")
        nc.sync.dma_start(bb[0:1, :], bib_d)
        nc.sync.dma_start(bb2[0:1, :], bhb_d)
        nc.vector.tensor_add(bb[0:1, :], bb[0:1, :], bb2[0:1, :])
        bbf = qtp.tile([1, G4], BF16, tag="bbf")
        nc.vector.tensor_copy(bbf[:, :], bb[0:1, :])
        ones8 = qtp.tile([1, BC], BF16, tag="ones8")
        nc.vector.memset(ones8[:, :], 1.0)
        for m24 in range(24):
            qp = qpp.tile([128, BC], F32)
            for k in range(6):
                nc.tensor.matmul(
                    qp[:, :],
                    lhsT=whb[:, k * G4 + m24 * 128: k * G4 + (m24 + 1) * 128],
                    rhs=ring_v[:, k, 1, :, UB - 1],
                    start=(k == 0), stop=False, skip_group_check=True)
            nc.tensor.matmul(qp[:, :], lhsT=bbf[0:1, m24 * 128:(m24 + 1) * 128],
                             rhs=ones8[0:1, :], start=False, stop=True, skip_group_check=True)
            nc.scalar.activation(qT[:, m24 * BC:(m24 + 1) * BC], qp[:, :], AF.Copy)

    # ---------------- phase 4: backward direction, fused per h-chunk ----------------
    with tc.tile_pool(name="wib", bufs=1) as wbp, \
         tc.tile_pool(name="gbps", bufs=1, space="PSUM") as gbpp, \
         tc.tile_pool(name="gbs", bufs=1) as gbsp:
        wib = wbp.tile([128, 4 * G4], BF16)
        load_cast_bf16(nc, tc, wib, wib_d, E, G4)
        HW2 = NBS // 2
        BH = BC // 2
        for m in range(6):
            for hf in range(2):
                def gb_mm(psum, gate):
                    col0 = gate * H + m * 128
                    for k in range(4):
                        for (n0, nw) in nchunks(HW2):
                            nc.tensor.matmul(
                                psum[:, n0:n0 + nw],
                                lhsT=wib[:, k * G4 + col0: k * G4 + col0 + 128],
                                rhs=embT[:, k * NBS + hf * HW2 + n0:
                                         k * NBS + hf * HW2 + n0 + nw],
                                start=(k == 0), stop=(k == 3), skip_group_check=True)

                def gb_act(dst, psum, gate, func):
                    m24 = gate * 6 + m
                    for bq in range(BH):
                        b = hf * BH + bq
                        nc.scalar.activation(
                            dst[:, bq * S:(bq + 1) * S],
                            psum[:, bq * S:(bq + 1) * S], func,
                            bias=qT[:, m24 * BC + b: m24 * BC + b + 1])

                psA = gbpp.tile([128, HW2], F32, tag="psA")
                psB = gbpp.tile([128, HW2], F32, tag="psB")
                gb_mm(psA, 0)          # r2
                gb_mm(psB, 2)          # g2
                sr = gbsp.tile([128, HW2], BF16, tag="sr")
                tg2 = gbsp.tile([128, HW2], BF16, tag="tg2")
                gb_act(sr, psA, 0, AF.Sigmoid)
                gb_act(tg2, psB, 2, AF.Tanh)
                u_sb = gbsp.tile([128, HW2], F32, tag="u")
                nc.vector.tensor_mul(u_sb[:, :], sr[:, :], tg2[:, :])
                psC = gbpp.tile([128, HW2], F32, tag="psA")
                psD = gbpp.tile([128, HW2], F32, tag="psB")
                gb_mm(psC, 1)          # f2
                gb_mm(psD, 3)          # o2
                f2s = gbsp.tile([128, HW2], F32, tag="f2s")
                o2s = gbsp.tile([128, HW2], BF16, tag="o2s")
                gb_act(f2s, psC, 1, AF.Sigmoid)
                gb_act(o2s, psD, 3, AF.Sigmoid)
                c2 = gbsp.tile([128, HW2], F32, tag="c2")
                for bq in range(BH):
                    sl = slice(bq * S, (bq + 1) * S)
                    nc.vector.tensor_tensor_scan(
                        c2[:, sl][:, ::-1], f2s[:, sl][:, ::-1],
                        u_sb[:, sl][:, ::-1], 0.0, ALU.mult, ALU.add)
                tc2 = gbsp.tile([128, HW2], BF16, tag="tc2")
                nc.scalar.activation(tc2[:, :], c2[:, :], AF.Tanh)
                hsb = gbsp.tile([128, HW2], BF16, tag="hsb")
                nc.vector.tensor_mul(hsb[:, :], o2s[:, :], tc2[:, :])
                nc.sync.dma_start(
                    hsbT_dram[m * 128:(m + 1) * 128, hf * HW2:(hf + 1) * HW2],
                    hsb[:, :])

    # ---------------- phase 5: output projection + pad bias ----------------
    with tc.tile_pool(name="prj", bufs=1) as pp, \
         tc.tile_pool(name="prjps", bufs=1, space="PSUM") as ppp, \
         tc.tile_pool(name="prjs", bufs=3) as psp:
        woutb = pp.tile([128, 12 * L], BF16)
        load_cast_bf16(nc, tc, woutb, wout_d, 2 * H, L)
        tok1 = pp.tile([1, NBS], I32)
        nc.sync.dma_start(tok1[0:1, :], tok_d.rearrange("(o n) -> o n", o=1))
        mask = pp.tile([1, NBS], F32)
        nc.vector.tensor_scalar(mask[0:1, :], tok1[0:1, :], PAD_WORD, None, ALU.is_equal)
        wmask = pp.tile([1, L], F32)
        nc.vector.memset(wmask[:, :], 0.0)
        nc.vector.memset(wmask[0:1, 0:1], PAD_BIAS)
        ones = pp.tile([1, NBS], F32)
        nc.vector.memset(ones[:, :], 1.0)
        boutf = pp.tile([1, L], F32)
        nc.sync.dma_start(boutf[0:1, :], bout_d)

        pproj = ppp.tile([L, NBS], F32)
        for kc in range(12):
            src = hsfT_dram if kc < 6 else hsbT_dram
            r0 = (kc % 6) * 128
            hs = psp.tile([128, NBS], BF16, tag="hs")
            nc.sync.dma_start(hs[:, :], src[r0:r0 + 128, :])
            if kc < 6:
                # stored col = blk*(BC*U) + b*U + u  ->  stream per-b (blk, u)
                hs_v = hs.rearrange("p (blk b u) -> p b blk u", b=BC, u=UB)
                for b in range(BC):
                    nc.tensor.matmul(pproj[:, b * S:(b + 1) * S],
                                     lhsT=woutb[:, kc * L:(kc + 1) * L],
                                     rhs=hs_v[:, b, :, :],
                                     start=(kc == 0), stop=False, skip_group_check=True)
            else:
                for (n0, nw) in nchunks(NBS):
                    nc.tensor.matmul(pproj[:, n0:n0 + nw],
                                     lhsT=woutb[:, kc * L:(kc + 1) * L],
                                     rhs=hs[:, n0:n0 + nw],
                                     start=False, stop=False, skip_group_check=True)
        for (n0, nw) in nchunks(NBS):
            nc.tensor.matmul(pproj[:, n0:n0 + nw], lhsT=wmask[0:1, :],
                             rhs=mask[0:1, n0:n0 + nw], start=False, stop=False, skip_group_check=True)
        for (n0, nw) in nchunks(NBS):
            nc.tensor.matmul(pproj[:, n0:n0 + nw], lhsT=boutf[0:1, :],
                             rhs=ones[0:1, n0:n0 + nw], start=False, stop=True, skip_group_check=True)
        outs = pp.tile([L, NBS], F32)
        nc.vector.tensor_copy(outs[:, :], pproj[:, :])
        nc.sync.dma_start(out_d, outs[:, :])

    estack.close()


def make_host_inputs(inputs, cfg, core):
    """Per-core in_map from full inputs (pure indexing / layout prep, no arithmetic)."""
    S = cfg["S"]
    perm = fwd_perm()
    toks = np.asarray(inputs["tokens"])[:S, core * BC:(core + 1) * BC]   # [S, BC]
    tokens_bm = np.ascontiguousarray(toks.T).reshape(-1).astype(np.int32)

    bf = ml_dtypes.bfloat16
    id128 = np.eye(128, dtype=bf)
    id8 = np.zeros((128, 8), dtype=bf)
    for q in range(4):
        for p in range(8):
            id8[32 * q + p, p] = 1
    f32 = lambda x: np.ascontiguousarray(np.asarray(x), dtype=np.float32)
    b16 = lambda x: np.ascontiguousarray(np.asarray(x), dtype=np.float32).astype(bf)
    uniq, inv = np.unique(tokens_bm, return_inverse=True)
    S_cfg = S
    NBS_ = BC * S_cfg
    table = np.zeros((NBS_, E), dtype=bf)
    table[:len(uniq)] = b16(np.asarray(inputs["embedding"], dtype=np.float32)[uniq])
    return {
        "tokens_bm": tokens_bm,
        "tokens_cm": inv.astype(np.int32),
        "embedding": table,
        "Wi_f_p": b16(np.asarray(inputs["Wi_f"], dtype=np.float32)[:, perm]),
        "Wh_f_p": b16(np.asarray(inputs["Wh_f"], dtype=np.float32)[:, perm]),
        "bi_f_p": f32(inputs["bi_f"])[perm].reshape(1, -1),
        "bh_f_p": f32(inputs["bh_f"])[perm].reshape(1, -1),
        "Wi_b": b16(inputs["Wi_b"]),
        "Wh_b": b16(inputs["Wh_b"]),
        "bi_b": f32(inputs["bi_b"]).reshape(1, -1),
        "bh_b": f32(inputs["bh_b"]).reshape(1, -1),
        "Wout": b16(inputs["Wout"]),
        "bout": f32(inputs["bout"]).reshape(1, -1),
        "id128": id128,
        "id8": id8,
    }


def assemble_output(results, cfg):
    S = cfg["S"]
    outs = []
    for r in results:
        o = np.asarray(r["outT"]).reshape(L, BC, S)        # [L, b, s]
        outs.append(np.transpose(o, (2, 1, 0)))            # [s, b, L]
    return np.ascontiguousarray(np.concatenate(outs, axis=1), dtype=np.float32)



def _split_excess_waits(raw: bytes, limit: int = 1) -> bytes:
    """Hoist excess per-instruction sem waits onto injected same-engine NoOps.

    walrus's setupSyncWait rejects engine/DMA instructions with too many sync
    waits; a NoOp on the same engine immediately before is semantically
    equivalent (the sequencer blocks either way) and NoOps lower fine with
    many waits (Tile's own kernel-tail drains carry 19+).
    """
    import orjson
    j = orjson.loads(raw)
    cnt = 0
    for fn in j.get("functions", []):
        for bb in fn.get("blocks") or []:
            insts = bb.get("instructions") or []
            out = []
            for inst in insts:
                si = inst.get("sync_info")
                waits = (si or {}).get("on_wait") or []
                if si and len(waits) > limit and inst.get("opcode") != "ISA":
                    excess, keep = waits[:-limit], waits[-limit:]
                    for w in excess:
                        nop = {
                            "engine": inst["engine"], "ins": [], "outs": [],
                            "name": f"waitsplit_{cnt}", "opcode": "EventSemaphore",
                            "sync_info": {"on_update": [], "on_wait": [w]},
                        }
                        if "debug" in inst:
                            nop["debug"] = inst["debug"]
                        out.append(nop)
                        cnt += 1
                    si["on_wait"] = keep
                out.append(inst)
            bb["instructions"] = out
    return orjson.dumps(j)

_BUILT = {}


def build_bass(num_devices=NCORES):
    if "full" in _BUILT:
        return _BUILT["full"]
    nc = bacc.Bacc("TRN2", target_bir_lowering=False, debug=False,
                   enable_asserts=False, num_devices=num_devices)
    with tile.TileContext(nc, pool_alloc_mode="queue") as tc:
        build_kernel(nc, tc, FULL_CFG)
    nc.compile()
    orig_to_json = nc.to_json_bytes
    nc.to_json_bytes = lambda: _split_excess_waits(orig_to_json())
    _BUILT["full"] = nc
    return nc


def run_on_hw(inputs, trace=False, trace_kwargs=None):
    from concourse import bass_utils
    nc = build_bass()
    in_maps = [make_host_inputs(inputs, FULL_CFG, core) for core in range(NCORES)]
    res = bass_utils.run_bass_kernel_spmd(
        nc, in_maps, core_ids=list(range(NCORES)),
        trace=trace, trace_kwargs=trace_kwargs or {})
    out = assemble_output(res.results, FULL_CFG)
    return out, res


def kernel(**inputs) -> np.ndarray:
    out, _ = run_on_hw(inputs, trace=False)
    return out


def run_timed(inputs, iters=3, pipeline_n=384):
    """Time NEFF executions with device-resident inputs (axon/PJRT path).

    Mirrors bass2jax.run_bass_via_pjrt's multi-core branch but device_puts the
    inputs once and loops the sharded call. The axon PJRT dispatch adds a fixed
    ~76-81ms round-trip LATENCY per synchronous call, but back-to-back
    dispatches pipeline: the device queue serializes the actual NEFF
    executions while the dispatch latency overlaps, so steady-state
    per-execution time = wall(N back-to-back executions)/N. Each round issues
    ``pipeline_n`` dispatches and blocks once at the end; we report the best
    round's per-execution average. Returns (best_seconds, all_times, out_full).
    """
    import time
    import jax
    import jax.numpy as jnp
    from jax.sharding import Mesh, PartitionSpec
    from jax.experimental.shard_map import shard_map
    from concourse import bass2jax, mybir as _mybir

    nc = build_bass()
    bass2jax.install_neuronx_cc_hook()
    in_maps = [make_host_inputs(inputs, FULL_CFG, core) for core in range(NCORES)]

    part_name = nc.partition_id_tensor.name if nc.partition_id_tensor else None
    in_names, out_names, out_avals, zero_outs = [], [], [], []
    for alloc in nc.m.functions[0].allocations:
        if not isinstance(alloc, _mybir.MemoryLocationSet):
            continue
        name = alloc.memorylocations[0].name
        if alloc.kind == "ExternalInput":
            if name != part_name:
                in_names.append(name)
        elif alloc.kind == "ExternalOutput":
            shape = tuple(alloc.tensor_shape)
            dtype = _mybir.dt.np(alloc.dtype)
            out_names.append(name)
            out_avals.append(jax.core.ShapedArray(shape, dtype))
            zero_outs.append(np.zeros(shape, dtype))
    n_params = len(in_names)
    all_in_names = in_names + out_names
    if part_name is not None:
        all_in_names = all_in_names + [part_name]

    def _body(*args):
        operands = list(args)
        if part_name is not None:
            operands.append(bass2jax.partition_id_tensor())
        outs = bass2jax._bass_exec_p.bind(
            *operands, out_avals=tuple(out_avals), in_names=tuple(all_in_names),
            out_names=tuple(out_names), lowering_input_output_aliases=(),
            sim_require_finite=True, sim_require_nnan=True, nc=nc)
        return tuple(outs)

    devices = jax.devices()[:NCORES]
    mesh = Mesh(np.asarray(devices), ("core",))
    n_outs = len(out_names)
    donate = tuple(range(n_params, n_params + n_outs))
    sharded = jax.jit(
        shard_map(_body, mesh=mesh,
                  in_specs=(PartitionSpec("core"),) * (n_params + n_outs),
                  out_specs=(PartitionSpec("core"),) * n_outs, check_rep=False),
        donate_argnums=donate, keep_unused=True)

    from jax.sharding import NamedSharding
    shard = NamedSharding(mesh, PartitionSpec("core"))
    concat_in = [
        jax.device_put(
            np.concatenate([np.asarray(in_maps[c][n]) for c in range(NCORES)], axis=0),
            shard)
        for n in in_names]
    # warmup (NEFF load + first-exec init)
    warm = [jax.device_put(
        np.zeros((NCORES * z.shape[0], *z.shape[1:]), z.dtype), shard)
        for z in zero_outs]
    out_arrs = sharded(*concat_in, *warm)
    for o in out_arrs:
        o.block_until_ready()

    times = []
    for it in range(iters):
        # donation consumes the zero output buffers: pre-make one set per
        # dispatch, all before the clock starts
        zsets = [[jax.device_put(
            np.zeros((NCORES * z.shape[0], *z.shape[1:]), z.dtype), shard)
            for z in zero_outs] for _ in range(pipeline_n)]
        for zs in zsets:
            for z in zs:
                z.block_until_ready()
        t0 = time.perf_counter()
        for zs in zsets:
            out_arrs = sharded(*concat_in, *zs)
        for o in out_arrs:
            o.block_until_ready()
        times.append((time.perf_counter() - t0) / pipeline_n)
    results = [
        {name: np.asarray(out_arrs[i]).reshape(NCORES, *out_avals[i].shape)[c]
         for i, name in enumerate(out_names)}
        for c in range(NCORES)]
    out = assemble_output(results, FULL_CFG)
    return min(times), times, out



# revision 25
# speedup vs baseline: 1.1870x; 1.1870x over previous
"""Trainium2 Bass kernel for nn_AttentionBiLSTM_Seqence.

Model (S=512 seq, B=64 batch, E=512 emb, H=768 hidden, V=32000 vocab, L=32 labels):
  emb = embedding[tokens]                                  [S,B,E]
  forward LSTM scan (512 sequential steps, gate order r,f,g,o):
      g = xf_t + h @ Wh_f + bh_f ; c = sig(f)*c + sig(r)*tanh(g); h = sig(o)*tanh(c)
  backward direction: gb = emb@Wi_b + bi_b + hT@Wh_b + bh_b (hT = final fwd hidden),
      then a LINEAR reverse scan c2 = sig(f2)*c2 + sig(r2)*tanh(g2), hs_b = sig(o2)*tanh(c2)
  out = [hs_f, hs_b] @ Wout + bout ; out[:,:,0] += 10000*(tokens==1)

Sharding: batch-data-parallel over 8 cores (8 batch rows each), no collectives.

Per-core performance structure:
  * forward scan (16-step unrolled For_i, PE branch-prefetch hint): 4-way column-tiled
    matmuls (tile_position) stream Wh through the PE concurrently; gates live in a
    "gappy" [104,768] PSUM layout (quarter q of H at partitions 32q..32q+8) so
    sigmoid/tanh batch into 2 ACT instructions; emb@Wi_f is folded into the scan
    as 4 extra wave chunks read from the resident embT (staged per step into a
    fixed [128,32] tile by a full-width SBUF->SBUF DMA; bias enters via a K=1
    ones-row matmul that also inits PSUM) -- these waves are independent of
    h(t-1) and fill the PE idle during the sig(o)->h->transpose tail;
    h returns to lhsT layout via small PE transposes staged over 4 rotating
    PSUM banks (same-bank accumulation-group restarts stall the PE ~1.3us), chunk m
    of step u-1 interleaved ahead of gf wave k=m of step u; gf/rt gate tiles are
    single-buffered (their ACT reads land mid-step), only the o-gate tile is
    double-buffered (its sigmoid races the next step's injection).
  * backward scan: one tensor_tensor_scan per (h-chunk, batch) tile along reversed s.
  * all big matmuls bf16 with fp32 PSUM accumulation; recurrence state fp32.
"""

import sys
import numpy as np

sys.path.insert(0, "/opt/trn_rl_repo")

import ml_dtypes

import concourse.bass as bass
import concourse.bacc as bacc
import concourse.mybir as mybir
import concourse.tile as tile
from concourse.bass import ds, ts

BF16 = mybir.dt.bfloat16
F32 = mybir.dt.float32
I32 = mybir.dt.int32
AF = mybir.ActivationFunctionType
ALU = mybir.AluOpType

# -------- problem constants --------
S_FULL, B_FULL, E, H, V_FULL, L = 512, 64, 512, 768, 32000, 32
NCORES = 8
BC = B_FULL // NCORES          # batch per core = 8
PAD_WORD, PAD_BIAS = 1, 10000.0
G4 = 4 * H                     # 3072
QH = H // 4                    # 192 channels per quarter
U = 32                         # scan unroll per For_i iteration
UB = 16                        # hsfT block size (ring DMA'd out per half)
GP = 3 * 32 + BC               # 104: partitions covering the 4 gappy quarters

FULL_CFG = dict(S=S_FULL, V=V_FULL)


def fwd_perm():
    """perm[new] = old for the forward 4H axis: 4 quarters x [g|f|r|o] (192 each).

    g first so the whole tanh gate sits in PSUM bank A (cols 0:512) and its
    activation can overlap the bank-B (cols 512:768) matmul waves; f early so
    c *= sig(f) also overlaps them.
    """
    perm = np.zeros(G4, dtype=np.int64)
    base_of = {2: 0, 1: QH, 0: 2 * QH, 3: 3 * QH}  # orig gate idx (r,f,g,o) -> base
    for q in range(4):
        for g_orig, base in base_of.items():
            for j in range(QH):
                perm[q * 768 + base + j] = g_orig * H + q * QH + j
    return perm


def nchunks(total, maxn=512):
    out, o = [], 0
    while o < total:
        w = min(maxn, total - o)
        out.append((o, w))
        o += w
    return out


def load_cast_bf16(nc, tc, dst, src_dram, rows, cols):
    """DMA a [rows, cols] bf16 DRAM weight into dst bf16 SBUF [128, (rows/128)*cols]."""
    for k in range(rows // 128):
        nc.sync.dma_start(dst[:, k * cols:(k + 1) * cols],
                          src_dram[128 * k:128 * (k + 1), :])


def build_kernel(nc, tc, cfg):
    S, V = cfg["S"], cfg["V"]
    NBS = BC * S               # rows of (b,s), b-major
    NT = NBS // 128
    ST = S // 128              # 128-row s-tiles per batch row
    assert S % U == 0 and S % 128 == 0

    # ---------------- I/O ----------------
    tok_d = nc.dram_tensor("tokens_bm", [NBS], I32, kind="ExternalInput").ap()
    tokc_d = nc.dram_tensor("tokens_cm", [NBS], I32, kind="ExternalInput").ap()
    emb_d = nc.dram_tensor("embedding", [NBS, E], BF16, kind="ExternalInput").ap()
    wif_d = nc.dram_tensor("Wi_f_p", [E, G4], BF16, kind="ExternalInput").ap()
    whf_d = nc.dram_tensor("Wh_f_p", [H, G4], BF16, kind="ExternalInput").ap()
    bif_d = nc.dram_tensor("bi_f_p", [1, G4], F32, kind="ExternalInput").ap()
    bhf_d = nc.dram_tensor("bh_f_p", [1, G4], F32, kind="ExternalInput").ap()
    wib_d = nc.dram_tensor("Wi_b", [E, G4], BF16, kind="ExternalInput").ap()
    whb_d = nc.dram_tensor("Wh_b", [H, G4], BF16, kind="ExternalInput").ap()
    bib_d = nc.dram_tensor("bi_b", [1, G4], F32, kind="ExternalInput").ap()
    bhb_d = nc.dram_tensor("bh_b", [1, G4], F32, kind="ExternalInput").ap()
    wout_d = nc.dram_tensor("Wout", [2 * H, L], BF16, kind="ExternalInput").ap()
    bout_d = nc.dram_tensor("bout", [1, L], F32, kind="ExternalInput").ap()
    id128_d = nc.dram_tensor("id128", [128, 128], BF16, kind="ExternalInput").ap()
    id8_d = nc.dram_tensor("id8", [128, 8], BF16, kind="ExternalInput").ap()
    out_d = nc.dram_tensor("outT", [L, NBS], F32, kind="ExternalOutput").ap()

    from contextlib import ExitStack
    estack = ExitStack()
    glob = estack.enter_context(tc.tile_pool(name="glob", bufs=1))
    dram = estack.enter_context(tc.tile_pool(name="dram", bufs=1, space="DRAM"))

    embT = glob.tile([128, 4 * NBS], BF16)        # emb^T: [E-chunk k][(b,s) col]
    ring = glob.tile([128, 48 * U], BF16)         # h^T staging: col = m*(BC*U) + b*U + u
    id8 = glob.tile([128, 8], BF16)
    id128 = glob.tile([128, 128], BF16)
    qT = glob.tile([128, 24 * BC], F32)           # backward per-(chunk,b) bias columns

    whp = estack.enter_context(tc.tile_pool(name="whf", bufs=1))
    scst = estack.enter_context(tc.tile_pool(name="scst", bufs=1))
    xflp = estack.enter_context(tc.tile_pool(name="xfl", bufs=6))
    wh = whp.tile([128, 6 * G4], BF16)
    c_sb = scst.tile([128, QH], F32)
    sig = scst.tile([128, 3 * QH], BF16)
    tg = scst.tile([128, QH], BF16)
    tmp = scst.tile([128, QH], F32)
    tc_ = scst.tile([128, QH], BF16)
    h_sb = scst.tile([128, QH], BF16)
    ring_v = ring.rearrange("p (m h b uu) -> p m h b uu", m=6, h=2, b=BC)

    hsfT_dram = dram.tile([6 * 128, NBS], BF16)   # row m*128+p = h-chan, col b*S+s
    hsbT_dram = dram.tile([6 * 128, NBS], BF16)

    nc.sync.dma_start(id8[:, :], id8_d)
    nc.sync.dma_start(id128[:, :], id128_d)

    # ---------------- phase 0: gather emb rows, transpose to embT ----------------
    with tc.tile_pool(name="gath", bufs=3) as gp, \
         tc.tile_pool(name="gathp", bufs=4, space="PSUM") as gpp, \
         tc.tile_pool(name="offs", bufs=1) as offp:
        tok_off = offp.tile([128, NT], I32)
        nc.sync.dma_start(tok_off[:, :], tokc_d.rearrange("(t p) -> p t", p=128))

        g_all = offp.tile([128, NT * E], BF16)
        for t in range(NT):
            nc.gpsimd.indirect_dma_start(
                out=g_all[:, t * E:(t + 1) * E], out_offset=None, in_=emb_d[:, :],
                in_offset=bass.IndirectOffsetOnAxis(ap=tok_off[:, t:t + 1], axis=0),
            )
        for t in range(NT):
            g_bf = g_all[:, t * E:(t + 1) * E]
            for e in range(4):
                pt = gpp.tile([128, 128], BF16)
                nc.tensor.transpose(pt[:, :], g_bf[:, 128 * e:128 * (e + 1)], id128[:, :])
                nc.scalar.activation(embT[:, e * NBS + t * 128: e * NBS + (t + 1) * 128],
                                     pt[:, :], AF.Copy)

    # ---------------- phase 2: forward LSTM scan ----------------
    # xf = emb @ Wi_f + bias is folded INTO the scan: per step, 4 extra wave
    # chunks contract embT (already resident, staged per-step into a fixed
    # [128, 32] tile via a full-width SBUF->SBUF DMA — the old [8,3072]
    # xf-row DMA wrote only 8 SBUF partitions, ~1/16 port width, ~1.5us/step)
    # and the bias enters via a K=1 ones-row matmul that also inits PSUM.
    # The emb waves don't depend on h(t-1), so they fill the PE idle while
    # the sig(o) -> h -> transpose chain of the previous step completes.
    with tc.tile_pool(name="scps", bufs=1, space="PSUM") as scps, \
         tc.tile_pool(name="wifp", bufs=1) as wifp:
        load_cast_bf16(nc, tc, wh, whf_d, H, G4)
        wif = wifp.tile([128, 4 * G4], BF16, name="wif")
        load_cast_bf16(nc, tc, wif, wif_d, E, G4)
        brow = whp.tile([1, G4], F32, name="brow")
        brow2 = whp.tile([1, G4], F32, name="brow2")
        nc.sync.dma_start(brow[0:1, :], bif_d)
        nc.sync.dma_start(brow2[0:1, :], bhf_d)
        nc.vector.tensor_add(brow[0:1, :], brow[0:1, :], brow2[0:1, :])
        biasf = whp.tile([1, G4], BF16, name="biasf")
        nc.vector.tensor_copy(biasf[:, :], brow[0:1, :])
        ones1 = whp.tile([1, BC], BF16, name="ones1")
        nc.vector.memset(ones1[:, :], 1.0)
        embT_v = embT.rearrange("p (j b s) -> p j b s", j=4, b=BC)
        # Separate PSUM tiles per gate group and parity: Tile serializes an
        # engine READ of a psum tile against PE WRITES to the same tile, so
        # each gate's activation can only overlap the later gates' matmul
        # waves if the gate groups live in different tiles.
        # Layout per quadrant (perm [g|f|r|o]): gf = cols 0:384, rt = 384:576,
        # ot = 576:768.
        # gf/rt single-buffered: their activations fire right after their own
        # wave groups (early in the step), long before the next step's
        # injection; only the o-gate tile needs parity (sig(o) is the step
        # tail, racing the next step's injection). The freed PSUM banks give
        # the h^T transpose staging a 4-deep rotation — consecutive
        # accumulation-group restarts on a just-drained PSUM bank stall the
        # PE ~1.3us (HW-measured), so spreading the per-step transposes over
        # 4 banks instead of 2 doubles the reuse gap.
        pgf = scps.tile([128, 384], F32, tag="pgf", name="pgf")
        prt = scps.tile([128, 192], F32, tag="prt", name="prt")
        pot = [scps.tile([128, 192], F32, tag=f"pot{j}", name=f"pot{j}") for j in range(2)]
        pts = [scps.tile([128, 24], BF16, tag=f"pt{j}", name=f"pt{j}") for j in range(4)]

        nc.vector.memset(c_sb[:, :], 0.0)
        nc.vector.memset(ring[:, :], 0.0)
        nc.vector.memset(pgf[:, :], 0.0)
        nc.vector.memset(prt[:, :], 0.0)
        for j in range(2):
            nc.vector.memset(pot[j][:, :], 0.0)

        # transposes chunk-major ([8,128] single-shot where the chunk
        # lies in one quarter); staging tiles rotate over 4 PSUM banks so a
        # bank's next accumulation-group restart is 4 chunk-slots away
        CHUNK_PIECES = (
            ((0, 0, 128, 0, 128),),                    # m=0: q0 cols 0:128
            ((0, 128, 64, 0, 64), (1, 0, 64, 64, 64)),  # m=1
            ((1, 64, 128, 0, 128),),                   # m=2
            ((2, 0, 128, 0, 128),),                    # m=3
            ((2, 128, 64, 0, 64), (3, 0, 64, 64, 64)),  # m=4
            ((3, 64, 128, 0, 128),),                   # m=5
        )

        def emit_transpose_chunk(u, m):
            ptm = pts[(6 * u + m) % 4]
            for (q, c0, cw, p0, pw) in CHUNK_PIECES[m]:
                nc.tensor.matmul(
                    ptm[p0:p0 + pw, 0:BC],
                    h_sb[32 * q:32 * q + BC, c0:c0 + cw],
                    id8[32 * q:32 * q + BC, :BC],
                    is_transpose=True, tile_position=(32 * q, p0),
                    skip_group_check=True)
            nc.vector.tensor_copy(ring_v[:, m, u // UB, :, u % UB],
                                  ptm[:, 0:BC])

        def emit_transposes(u):
            for m in range(6):
                emit_transpose_chunk(u, m)

        # hsfT_dram column layout: col = (s//U)*(BC*U) + b*U + (s%U)
        with tc.For_i(0, S, U, hint_engines=(mybir.EngineType.PE, mybir.EngineType.DVE)) as i:
            for u in range(U):
                ggf = pgf
                grt = prt
                got = pot[u % 2]
                prev = (u - 1) % U
                # stage embT columns for this step (and prefetch next):
                # xe[:, j*8+b] = embT[:, j*NBS + b*S + (i+u)]
                if u == 0:
                    xe0 = xflp.tile([128, 4 * BC], BF16, tag="xe", name="xe0")
                    nc.sync.dma_start(xe0[:, :], embT_v[:, :, :, ds(i, 1)])
                    xes = {0: xe0}
                if u < U - 1:
                    xen = xflp.tile([128, 4 * BC], BF16, tag="xe", name=f"xe{u + 1}")
                    (nc.gpsimd if u % 2 == 0 else nc.scalar).dma_start(
                        xen[:, :], embT_v[:, :, :, ds(i + u + 1, 1)])
                    xes[u + 1] = xen
                xe = xes.pop(u)
                # bias K=1 injection (inits PSUM), then the 4 emb wave chunks;
                # neither depends on h(u-1)
                for dst, c0, cw in ((ggf, 0, 384), (grt, 384, 192), (got, 576, 192)):
                    for q in range(4):
                        nc.tensor.matmul(
                            dst[32 * q:32 * q + BC, :],
                            lhsT=ones1[0:1, 0:BC],
                            rhs=biasf[0:1, q * 768 + c0: q * 768 + c0 + cw],
                            start=True, stop=False, tile_position=(0, 32 * q),
                            skip_group_check=True)
                for dst, c0, cw in ((ggf, 0, 384), (grt, 384, 192), (got, 576, 192)):
                    for j in range(4):
                        for q in range(4):
                            nc.tensor.matmul(
                                dst[32 * q:32 * q + BC, :],
                                lhsT=xe[:, j * BC:(j + 1) * BC],
                                rhs=wif[:, j * G4 + q * 768 + c0: j * G4 + q * 768 + c0 + cw],
                                start=False, stop=False, tile_position=(0, 32 * q),
                                skip_group_check=True)
                # k-waves gate-group major: g+f first, then r, then o. Each
                # group's activation fires as soon as its own tile's waves end
                # and overlaps the later groups' waves; only sigmoid(o) and
                # h = sig(o)*tanh(c) remain exposed after the last wave.
                # Step u-1's transpose chunk m is interleaved just before the
                # gf wave k=m of step u: only chunk 0's transpose+ring-copy is
                # chain-exposed after h(u-1); chunks 1-5 hide inside the wave
                # stream (wave k only needs ring chunk k).
                for dst, c0, cw in ((ggf, 0, 384), (grt, 384, 192), (got, 576, 192)):
                    for k in range(6):
                        if dst is ggf and u > 0:
                            emit_transpose_chunk(u - 1, k)
                        for q in range(4):
                            nc.tensor.matmul(
                                dst[32 * q:32 * q + BC, :],
                                lhsT=ring_v[:, k, prev // UB, :, prev % UB],
                                rhs=wh[:, k * G4 + q * 768 + c0: k * G4 + q * 768 + c0 + cw],
                                start=False, stop=(k == 5), tile_position=(0, 32 * q),
                                skip_group_check=True)
                    if dst is ggf:
                        if u == UB:
                            # first half of the ring is complete: stream it
                            # out now so the DMAs overlap the second half
                            for m in range(6):
                                nc.gpsimd.dma_start(
                                    hsfT_dram[m * 128:(m + 1) * 128, ds(i * BC, 128)],
                                    ring[:, m * (BC * U): m * (BC * U) + 128])
                        nc.scalar.activation(tg[0:GP, :], ggf[0:GP, 0:QH], AF.Tanh)
                        nc.scalar.activation(sig[0:GP, 0:QH], ggf[0:GP, QH:2 * QH],
                                             AF.Sigmoid)
                    elif dst is grt:
                        nc.scalar.activation(sig[0:GP, QH:2 * QH], grt[0:GP, :],
                                             AF.Sigmoid)
                nc.scalar.activation(sig[0:GP, 2 * QH:3 * QH], got[0:GP, :], AF.Sigmoid)
                # sig semantic layout: [f 0:192 | r 192:384 | o 384:576]
                nc.vector.tensor_mul(c_sb[0:GP, :], c_sb[0:GP, :], sig[0:GP, 0:QH])
                nc.vector.tensor_mul(tmp[0:GP, :], sig[0:GP, QH:2 * QH], tg[0:GP, :])
                nc.vector.tensor_add(c_sb[0:GP, :], c_sb[0:GP, :], tmp[0:GP, :])
                nc.scalar.activation(tc_[0:GP, :], c_sb[0:GP, :], AF.Tanh)
                nc.vector.tensor_mul(h_sb[0:GP, :], sig[0:GP, 2 * QH:3 * QH], tc_[0:GP, :])
            emit_transposes(U - 1)
            for m in range(6):
                nc.gpsimd.dma_start(
                    hsfT_dram[m * 128:(m + 1) * 128, ds(i * BC + 128, 128)],
                    ring[:, m * (BC * U) + 128: m * (BC * U) + 256])

    # ---------------- phase 3: qT = Wh_b^T @ hT + (bi_b + bh_b) ----------------
    with tc.tile_pool(name="whb", bufs=1) as qwp, \
         tc.tile_pool(name="qps", bufs=4, space="PSUM") as qpp, \
         tc.tile_pool(name="qtmp", bufs=1) as qtp:
        whb = qwp.tile([128, 6 * G4], BF16)
        load_cast_bf16(nc, tc, whb, whb_d, H, G4)
        bb = qtp.tile([1, G4], F32, tag="bb")
        bb2 = qtp.tile([1, G4], F32, tag="bb2")
        nc.sync.dma_start(bb[0:1, :], bib_d)
        nc.sync.dma_start(bb2[0:1, :], bhb_d)
        nc.vector.tensor_add(bb[0:1, :], bb[0:1, :], bb2[0:1, :])
        bbf = qtp.tile([1, G4], BF16, tag="bbf")
        nc.vector.tensor_copy(bbf[:, :], bb[0:1, :])
        ones8 = qtp.tile([1, BC], BF16, tag="ones8")
        nc.vector.memset(ones8[:, :], 1.0)
        for m24 in range(24):
            qp = qpp.tile([128, BC], F32)
            for k in range(6):
                nc.tensor.matmul(
                    qp[:, :],
                    lhsT=whb[:, k * G4 + m24 * 128: k * G4 + (m24 + 1) * 128],
                    rhs=ring_v[:, k, 1, :, UB - 1],
                    start=(k == 0), stop=False, skip_group_check=True)
            nc.tensor.matmul(qp[:, :], lhsT=bbf[0:1, m24 * 128:(m24 + 1) * 128],
                             rhs=ones8[0:1, :], start=False, stop=True, skip_group_check=True)
            nc.scalar.activation(qT[:, m24 * BC:(m24 + 1) * BC], qp[:, :], AF.Copy)

    # ---------------- phase 4: backward direction, fused per h-chunk ----------------
    with tc.tile_pool(name="wib", bufs=1) as wbp, \
         tc.tile_pool(name="gbps", bufs=1, space="PSUM") as gbpp, \
         tc.tile_pool(name="gbs", bufs=1) as gbsp:
        wib = wbp.tile([128, 4 * G4], BF16)
        load_cast_bf16(nc, tc, wib, wib_d, E, G4)
        HW2 = NBS // 2
        BH = BC // 2
        for m in range(6):
            for hf in range(2):
                def gb_mm(psum, gate):
                    col0 = gate * H + m * 128
                    for k in range(4):
                        for (n0, nw) in nchunks(HW2):
                            nc.tensor.matmul(
                                psum[:, n0:n0 + nw],
                                lhsT=wib[:, k * G4 + col0: k * G4 + col0 + 128],
                                rhs=embT[:, k * NBS + hf * HW2 + n0:
                                         k * NBS + hf * HW2 + n0 + nw],
                                start=(k == 0), stop=(k == 3), skip_group_check=True)

                def gb_act(dst, psum, gate, func):
                    m24 = gate * 6 + m
                    for bq in range(BH):
                        b = hf * BH + bq
                        nc.scalar.activation(
                            dst[:, bq * S:(bq + 1) * S],
                            psum[:, bq * S:(bq + 1) * S], func,
                            bias=qT[:, m24 * BC + b: m24 * BC + b + 1])

                psA = gbpp.tile([128, HW2], F32, tag="psA")
                psB = gbpp.tile([128, HW2], F32, tag="psB")
                gb_mm(psA, 0)          # r2
                gb_mm(psB, 2)          # g2
                sr = gbsp.tile([128, HW2], BF16, tag="sr")
                tg2 = gbsp.tile([128, HW2], BF16, tag="tg2")
                gb_act(sr, psA, 0, AF.Sigmoid)
                gb_act(tg2, psB, 2, AF.Tanh)
                u_sb = gbsp.tile([128, HW2], F32, tag="u")
                nc.vector.tensor_mul(u_sb[:, :], sr[:, :], tg2[:, :])
                psC = gbpp.tile([128, HW2], F32, tag="psA")
                psD = gbpp.tile([128, HW2], F32, tag="psB")
                gb_mm(psC, 1)          # f2
                gb_mm(psD, 3)          # o2
                f2s = gbsp.tile([128, HW2], F32, tag="f2s")
                o2s = gbsp.tile([128, HW2], BF16, tag="o2s")
                gb_act(f2s, psC, 1, AF.Sigmoid)
                gb_act(o2s, psD, 3, AF.Sigmoid)
                c2 = gbsp.tile([128, HW2], F32, tag="c2")
                for bq in range(BH):
                    sl = slice(bq * S, (bq + 1) * S)
                    nc.vector.tensor_tensor_scan(
                        c2[:, sl][:, ::-1], f2s[:, sl][:, ::-1],
                        u_sb[:, sl][:, ::-1], 0.0, ALU.mult, ALU.add)
                tc2 = gbsp.tile([128, HW2], BF16, tag="tc2")
                nc.scalar.activation(tc2[:, :], c2[:, :], AF.Tanh)
                hsb = gbsp.tile([128, HW2], BF16, tag="hsb")
                nc.vector.tensor_mul(hsb[:, :], o2s[:, :], tc2[:, :])
                nc.sync.dma_start(
                    hsbT_dram[m * 128:(m + 1) * 128, hf * HW2:(hf + 1) * HW2],
                    hsb[:, :])

    # ---------------- phase 5: output projection + pad bias ----------------
    with tc.tile_pool(name="prj", bufs=1) as pp, \
         tc.tile_pool(name="prjps", bufs=1, space="PSUM") as ppp, \
         tc.tile_pool(name="prjs", bufs=3) as psp:
        woutb = pp.tile([128, 12 * L], BF16)
        load_cast_bf16(nc, tc, woutb, wout_d, 2 * H, L)
        tok1 = pp.tile([1, NBS], I32)
        nc.sync.dma_start(tok1[0:1, :], tok_d.rearrange("(o n) -> o n", o=1))
        mask = pp.tile([1, NBS], F32)
        nc.vector.tensor_scalar(mask[0:1, :], tok1[0:1, :], PAD_WORD, None, ALU.is_equal)
        wmask = pp.tile([1, L], F32)
        nc.vector.memset(wmask[:, :], 0.0)
        nc.vector.memset(wmask[0:1, 0:1], PAD_BIAS)
        ones = pp.tile([1, NBS], F32)
        nc.vector.memset(ones[:, :], 1.0)
        boutf = pp.tile([1, L], F32)
        nc.sync.dma_start(boutf[0:1, :], bout_d)

        pproj = ppp.tile([L, NBS], F32)
        for kc in range(12):
            src = hsfT_dram if kc < 6 else hsbT_dram
            r0 = (kc % 6) * 128
            hs = psp.tile([128, NBS], BF16, tag="hs")
            nc.sync.dma_start(hs[:, :], src[r0:r0 + 128, :])
            if kc < 6:
                # stored col = blk*(BC*U) + b*U + u  ->  stream per-b (blk, u)
                hs_v = hs.rearrange("p (blk b u) -> p b blk u", b=BC, u=UB)
                for b in range(BC):
                    nc.tensor.matmul(pproj[:, b * S:(b + 1) * S],
                                     lhsT=woutb[:, kc * L:(kc + 1) * L],
                                     rhs=hs_v[:, b, :, :],
                                     start=(kc == 0), stop=False, skip_group_check=True)
            else:
                for (n0, nw) in nchunks(NBS):
                    nc.tensor.matmul(pproj[:, n0:n0 + nw],
                                     lhsT=woutb[:, kc * L:(kc + 1) * L],
                                     rhs=hs[:, n0:n0 + nw],
                                     start=False, stop=False, skip_group_check=True)
        for (n0, nw) in nchunks(NBS):
            nc.tensor.matmul(pproj[:, n0:n0 + nw], lhsT=wmask[0:1, :],
                             rhs=mask[0:1, n0:n0 + nw], start=False, stop=False, skip_group_check=True)
        for (n0, nw) in nchunks(NBS):
            nc.tensor.matmul(pproj[:, n0:n0 + nw], lhsT=boutf[0:1, :],
                             rhs=ones[0:1, n0:n0 + nw], start=False, stop=True, skip_group_check=True)
        outs = pp.tile([L, NBS], F32)
        nc.vector.tensor_copy(outs[:, :], pproj[:, :])
        nc.sync.dma_start(out_d, outs[:, :])

    estack.close()


def make_host_inputs(inputs, cfg, core):
    """Per-core in_map from full inputs (pure indexing / layout prep, no arithmetic)."""
    S = cfg["S"]
    perm = fwd_perm()
    toks = np.asarray(inputs["tokens"])[:S, core * BC:(core + 1) * BC]   # [S, BC]
    tokens_bm = np.ascontiguousarray(toks.T).reshape(-1).astype(np.int32)

    bf = ml_dtypes.bfloat16
    id128 = np.eye(128, dtype=bf)
    id8 = np.zeros((128, 8), dtype=bf)
    for q in range(4):
        for p in range(8):
            id8[32 * q + p, p] = 1
    f32 = lambda x: np.ascontiguousarray(np.asarray(x), dtype=np.float32)
    b16 = lambda x: np.ascontiguousarray(np.asarray(x), dtype=np.float32).astype(bf)
    uniq, inv = np.unique(tokens_bm, return_inverse=True)
    S_cfg = S
    NBS_ = BC * S_cfg
    table = np.zeros((NBS_, E), dtype=bf)
    table[:len(uniq)] = b16(np.asarray(inputs["embedding"], dtype=np.float32)[uniq])
    return {
        "tokens_bm": tokens_bm,
        "tokens_cm": inv.astype(np.int32),
        "embedding": table,
        "Wi_f_p": b16(np.asarray(inputs["Wi_f"], dtype=np.float32)[:, perm]),
        "Wh_f_p": b16(np.asarray(inputs["Wh_f"], dtype=np.float32)[:, perm]),
        "bi_f_p": f32(inputs["bi_f"])[perm].reshape(1, -1),
        "bh_f_p": f32(inputs["bh_f"])[perm].reshape(1, -1),
        "Wi_b": b16(inputs["Wi_b"]),
        "Wh_b": b16(inputs["Wh_b"]),
        "bi_b": f32(inputs["bi_b"]).reshape(1, -1),
        "bh_b": f32(inputs["bh_b"]).reshape(1, -1),
        "Wout": b16(inputs["Wout"]),
        "bout": f32(inputs["bout"]).reshape(1, -1),
        "id128": id128,
        "id8": id8,
    }


def assemble_output(results, cfg):
    S = cfg["S"]
    outs = []
    for r in results:
        o = np.asarray(r["outT"]).reshape(L, BC, S)        # [L, b, s]
        outs.append(np.transpose(o, (2, 1, 0)))            # [s, b, L]
    return np.ascontiguousarray(np.concatenate(outs, axis=1), dtype=np.float32)



def _split_excess_waits(raw: bytes, limit: int = 1) -> bytes:
    """Hoist excess per-instruction sem waits onto injected same-engine NoOps.

    walrus's setupSyncWait rejects engine/DMA instructions with too many sync
    waits; a NoOp on the same engine immediately before is semantically
    equivalent (the sequencer blocks either way) and NoOps lower fine with
    many waits (Tile's own kernel-tail drains carry 19+).
    """
    import orjson
    j = orjson.loads(raw)
    cnt = 0
    for fn in j.get("functions", []):
        for bb in fn.get("blocks") or []:
            insts = bb.get("instructions") or []
            out = []
            for inst in insts:
                si = inst.get("sync_info")
                waits = (si or {}).get("on_wait") or []
                if si and len(waits) > limit and inst.get("opcode") != "ISA":
                    excess, keep = waits[:-limit], waits[-limit:]
                    for w in excess:
                        nop = {
                            "engine": inst["engine"], "ins": [], "outs": [],
                            "name": f"waitsplit_{cnt}", "opcode": "EventSemaphore",
                            "sync_info": {"on_update": [], "on_wait": [w]},
                        }
                        if "debug" in inst:
                            nop["debug"] = inst["debug"]
                        out.append(nop)
                        cnt += 1
                    si["on_wait"] = keep
                out.append(inst)
            bb["instructions"] = out
    return orjson.dumps(j)

_BUILT = {}


def build_bass(num_devices=NCORES):
    if "full" in _BUILT:
        return _BUILT["full"]
    nc = bacc.Bacc("TRN2", target_bir_lowering=False, debug=False,
                   enable_asserts=False, num_devices=num_devices)
    with tile.TileContext(nc, pool_alloc_mode="queue") as tc:
        build_kernel(nc, tc, FULL_CFG)
    nc.compile()
    orig_to_json = nc.to_json_bytes
    nc.to_json_bytes = lambda: _split_excess_waits(orig_to_json())
    _BUILT["full"] = nc
    return nc


def run_on_hw(inputs, trace=False, trace_kwargs=None):
    from concourse import bass_utils
    nc = build_bass()
    in_maps = [make_host_inputs(inputs, FULL_CFG, core) for core in range(NCORES)]
    res = bass_utils.run_bass_kernel_spmd(
        nc, in_maps, core_ids=list(range(NCORES)),
        trace=trace, trace_kwargs=trace_kwargs or {})
    out = assemble_output(res.results, FULL_CFG)
    return out, res


def kernel(**inputs) -> np.ndarray:
    out, _ = run_on_hw(inputs, trace=False)
    return out


def run_timed(inputs, iters=3, pipeline_n=384):
    """Time NEFF executions with device-resident inputs (axon/PJRT path).

    Mirrors bass2jax.run_bass_via_pjrt's multi-core branch but device_puts the
    inputs once and loops the sharded call. The axon PJRT dispatch adds a fixed
    ~76-81ms round-trip LATENCY per synchronous call, but back-to-back
    dispatches pipeline: the device queue serializes the actual NEFF
    executions while the dispatch latency overlaps, so steady-state
    per-execution time = wall(N back-to-back executions)/N. Each round issues
    ``pipeline_n`` dispatches and blocks once at the end; we report the best
    round's per-execution average. Returns (best_seconds, all_times, out_full).
    """
    import time
    import jax
    import jax.numpy as jnp
    from jax.sharding import Mesh, PartitionSpec
    from jax.experimental.shard_map import shard_map
    from concourse import bass2jax, mybir as _mybir

    nc = build_bass()
    bass2jax.install_neuronx_cc_hook()
    in_maps = [make_host_inputs(inputs, FULL_CFG, core) for core in range(NCORES)]

    part_name = nc.partition_id_tensor.name if nc.partition_id_tensor else None
    in_names, out_names, out_avals, zero_outs = [], [], [], []
    for alloc in nc.m.functions[0].allocations:
        if not isinstance(alloc, _mybir.MemoryLocationSet):
            continue
        name = alloc.memorylocations[0].name
        if alloc.kind == "ExternalInput":
            if name != part_name:
                in_names.append(name)
        elif alloc.kind == "ExternalOutput":
            shape = tuple(alloc.tensor_shape)
            dtype = _mybir.dt.np(alloc.dtype)
            out_names.append(name)
            out_avals.append(jax.core.ShapedArray(shape, dtype))
            zero_outs.append(np.zeros(shape, dtype))
    n_params = len(in_names)
    all_in_names = in_names + out_names
    if part_name is not None:
        all_in_names = all_in_names + [part_name]

    def _body(*args):
        operands = list(args)
        if part_name is not None:
            operands.append(bass2jax.partition_id_tensor())
        outs = bass2jax._bass_exec_p.bind(
            *operands, out_avals=tuple(out_avals), in_names=tuple(all_in_names),
            out_names=tuple(out_names), lowering_input_output_aliases=(),
            sim_require_finite=True, sim_require_nnan=True, nc=nc)
        return tuple(outs)

    devices = jax.devices()[:NCORES]
    mesh = Mesh(np.asarray(devices), ("core",))
    n_outs = len(out_names)
    donate = tuple(range(n_params, n_params + n_outs))
    sharded = jax.jit(
        shard_map(_body, mesh=mesh,
                  in_specs=(PartitionSpec("core"),) * (n_params + n_outs),
                  out_specs=(PartitionSpec("core"),) * n_outs, check_rep=False),
        donate_argnums=donate, keep_unused=True)

    from jax.sharding import NamedSharding
    shard = NamedSharding(mesh, PartitionSpec("core"))
    concat_in = [
        jax.device_put(
            np.concatenate([np.asarray(in_maps[c][n]) for c in range(NCORES)], axis=0),
            shard)
        for n in in_names]
    # warmup (NEFF load + first-exec init)
    warm = [jax.device_put(
        np.zeros((NCORES * z.shape[0], *z.shape[1:]), z.dtype), shard)
        for z in zero_outs]
    out_arrs = sharded(*concat_in, *warm)
    for o in out_arrs:
        o.block_until_ready()

    times = []
    for it in range(iters):
        # donation consumes the zero output buffers: pre-make one set per
        # dispatch, all before the clock starts
        zsets = [[jax.device_put(
            np.zeros((NCORES * z.shape[0], *z.shape[1:]), z.dtype), shard)
            for z in zero_outs] for _ in range(pipeline_n)]
        for zs in zsets:
            for z in zs:
                z.block_until_ready()
        t0 = time.perf_counter()
        for zs in zsets:
            out_arrs = sharded(*concat_in, *zs)
        for o in out_arrs:
            o.block_until_ready()
        times.append((time.perf_counter() - t0) / pipeline_n)
    results = [
        {name: np.asarray(out_arrs[i]).reshape(NCORES, *out_avals[i].shape)[c]
         for i, name in enumerate(out_names)}
        for c in range(NCORES)]
    out = assemble_output(results, FULL_CFG)
    return min(times), times, out



# revision 26
# speedup vs baseline: 1.4050x; 1.1837x over previous
"""Trainium2 Bass kernel for nn_AttentionBiLSTM_Seqence.

Model (S=512 seq, B=64 batch, E=512 emb, H=768 hidden, V=32000 vocab, L=32 labels):
  emb = embedding[tokens]                                  [S,B,E]
  forward LSTM scan (512 sequential steps, gate order r,f,g,o):
      g = xf_t + h @ Wh_f + bh_f ; c = sig(f)*c + sig(r)*tanh(g); h = sig(o)*tanh(c)
  backward direction: gb = emb@Wi_b + bi_b + hT@Wh_b + bh_b (hT = final fwd hidden),
      then a LINEAR reverse scan c2 = sig(f2)*c2 + sig(r2)*tanh(g2), hs_b = sig(o2)*tanh(c2)
  out = [hs_f, hs_b] @ Wout + bout ; out[:,:,0] += 10000*(tokens==1)

Sharding: batch-data-parallel over 8 cores (8 batch rows each), no collectives.

Per-core performance structure:
  * forward scan (16-step unrolled For_i, PE branch-prefetch hint): 4-way column-tiled
    matmuls (tile_position) stream Wh through the PE concurrently; gates live in a
    "gappy" [104,768] PSUM layout (quarter q of H at partitions 32q..32q+8) so
    sigmoid/tanh batch into 2 ACT instructions; emb@Wi_f is folded into the scan
    as 4 extra wave chunks read from the resident embT (staged per step into a
    fixed [128,32] tile by a full-width SBUF->SBUF DMA; bias enters via a K=1
    ones-row matmul that also inits PSUM) -- these waves are independent of
    h(t-1) and fill the PE idle during the sig(o)->h->transpose tail;
    h returns to lhsT layout via small PE transposes staged over 4 rotating
    PSUM banks (same-bank accumulation-group restarts stall the PE ~1.3us), chunk m
    of step u-1 interleaved ahead of gf wave k=m of step u; gf/rt gate tiles are
    single-buffered (their ACT reads land mid-step), only the o-gate tile is
    double-buffered (its sigmoid races the next step's injection).
  * backward scan: one tensor_tensor_scan per (h-chunk, batch) tile along reversed s.
  * all big matmuls bf16 with fp32 PSUM accumulation; recurrence state fp32.
"""

import sys
import numpy as np

sys.path.insert(0, "/opt/trn_rl_repo")

import ml_dtypes

import concourse.bass as bass
import concourse.bacc as bacc
import concourse.mybir as mybir
import concourse.tile as tile
from concourse.bass import ds, ts

BF16 = mybir.dt.bfloat16
F32 = mybir.dt.float32
I32 = mybir.dt.int32
AF = mybir.ActivationFunctionType
ALU = mybir.AluOpType

# -------- problem constants --------
S_FULL, B_FULL, E, H, V_FULL, L = 512, 64, 512, 768, 32000, 32
NCORES = 8
BC = B_FULL // NCORES          # batch per core = 8
PAD_WORD, PAD_BIAS = 1, 10000.0
G4 = 4 * H                     # 3072
QH = H // 4                    # 192 channels per quarter
U = 32                         # scan unroll per For_i iteration
UB = 16                        # hsfT block size (ring DMA'd out per half)
GP = 3 * 32 + BC               # 104: partitions covering the 4 gappy quarters

FULL_CFG = dict(S=S_FULL, V=V_FULL)


def fwd_perm():
    """perm[new] = old for the forward 4H axis: 4 quarters x [g|f|r|o] (192 each).

    g first so the whole tanh gate sits in PSUM bank A (cols 0:512) and its
    activation can overlap the bank-B (cols 512:768) matmul waves; f early so
    c *= sig(f) also overlaps them.
    """
    perm = np.zeros(G4, dtype=np.int64)
    base_of = {2: 0, 1: QH, 0: 2 * QH, 3: 3 * QH}  # orig gate idx (r,f,g,o) -> base
    for q in range(4):
        for g_orig, base in base_of.items():
            for j in range(QH):
                perm[q * 768 + base + j] = g_orig * H + q * QH + j
    return perm


def nchunks(total, maxn=512):
    out, o = [], 0
    while o < total:
        w = min(maxn, total - o)
        out.append((o, w))
        o += w
    return out


def load_cast_bf16(nc, tc, dst, src_dram, rows, cols):
    """DMA a [rows, cols] bf16 DRAM weight into dst bf16 SBUF [128, (rows/128)*cols]."""
    for k in range(rows // 128):
        nc.sync.dma_start(dst[:, k * cols:(k + 1) * cols],
                          src_dram[128 * k:128 * (k + 1), :])


def build_kernel(nc, tc, cfg):
    S, V = cfg["S"], cfg["V"]
    NBS = BC * S               # rows of (b,s), b-major
    NT = NBS // 128
    ST = S // 128              # 128-row s-tiles per batch row
    assert S % U == 0 and S % 128 == 0

    # ---------------- I/O ----------------
    tok_d = nc.dram_tensor("tokens_bm", [NBS], I32, kind="ExternalInput").ap()
    tokc_d = nc.dram_tensor("tokens_cm", [NBS], I32, kind="ExternalInput").ap()
    emb_d = nc.dram_tensor("embedding", [NBS, E], BF16, kind="ExternalInput").ap()
    wif_d = nc.dram_tensor("Wi_f_p", [E, G4], BF16, kind="ExternalInput").ap()
    whf_d = nc.dram_tensor("Wh_f_p", [H, G4], BF16, kind="ExternalInput").ap()
    bif_d = nc.dram_tensor("bi_f_p", [1, G4], F32, kind="ExternalInput").ap()
    bhf_d = nc.dram_tensor("bh_f_p", [1, G4], F32, kind="ExternalInput").ap()
    wib_d = nc.dram_tensor("Wi_b", [E, G4], BF16, kind="ExternalInput").ap()
    whb_d = nc.dram_tensor("Wh_b", [H, G4], BF16, kind="ExternalInput").ap()
    bib_d = nc.dram_tensor("bi_b", [1, G4], F32, kind="ExternalInput").ap()
    bhb_d = nc.dram_tensor("bh_b", [1, G4], F32, kind="ExternalInput").ap()
    wout_d = nc.dram_tensor("Wout", [2 * H, L], BF16, kind="ExternalInput").ap()
    bout_d = nc.dram_tensor("bout", [1, L], F32, kind="ExternalInput").ap()
    id128_d = nc.dram_tensor("id128", [128, 128], BF16, kind="ExternalInput").ap()
    id8_d = nc.dram_tensor("id8", [128, 8], BF16, kind="ExternalInput").ap()
    out_d = nc.dram_tensor("outT", [L, NBS], F32, kind="ExternalOutput").ap()

    from contextlib import ExitStack
    estack = ExitStack()
    glob = estack.enter_context(tc.tile_pool(name="glob", bufs=1))
    dram = estack.enter_context(tc.tile_pool(name="dram", bufs=1, space="DRAM"))

    embT = glob.tile([128, 4 * NBS], BF16)        # emb^T: [E-chunk k][(b,s) col]
    ring = glob.tile([128, 48 * U], BF16)         # h^T staging: col = m*(BC*U) + b*U + u
    id8 = glob.tile([128, 8], BF16)
    id128 = glob.tile([128, 128], BF16)
    qT = glob.tile([128, 24 * BC], F32)           # backward per-(chunk,b) bias columns

    whp = estack.enter_context(tc.tile_pool(name="whf", bufs=1))
    scst = estack.enter_context(tc.tile_pool(name="scst", bufs=1))
    xflp = estack.enter_context(tc.tile_pool(name="xfl", bufs=6))
    wh = whp.tile([128, 6 * G4], BF16)
    c_sb = scst.tile([128, QH], F32)
    sig = scst.tile([128, 3 * QH], BF16)
    tg = scst.tile([128, QH], BF16)
    tmp = scst.tile([128, QH], F32)
    tc_ = scst.tile([128, QH], BF16)
    h_sb = scst.tile([128, QH], BF16)
    ring_v = ring.rearrange("p (m h b uu) -> p m h b uu", m=6, h=2, b=BC)

    hsfT_dram = dram.tile([6 * 128, NBS], BF16)   # row m*128+p = h-chan, col b*S+s
    hsbT_dram = dram.tile([6 * 128, NBS], BF16)

    nc.sync.dma_start(id8[:, :], id8_d)
    nc.sync.dma_start(id128[:, :], id128_d)

    # ---------------- phase 0: gather emb rows, transpose to embT ----------------
    with tc.tile_pool(name="gath", bufs=3) as gp, \
         tc.tile_pool(name="gathp", bufs=4, space="PSUM") as gpp, \
         tc.tile_pool(name="offs", bufs=1) as offp:
        tok_off = offp.tile([128, NT], I32)
        nc.sync.dma_start(tok_off[:, :], tokc_d.rearrange("(t p) -> p t", p=128))

        g_all = offp.tile([128, NT * E], BF16)
        for t in range(NT):
            nc.gpsimd.indirect_dma_start(
                out=g_all[:, t * E:(t + 1) * E], out_offset=None, in_=emb_d[:, :],
                in_offset=bass.IndirectOffsetOnAxis(ap=tok_off[:, t:t + 1], axis=0),
            )
        for t in range(NT):
            g_bf = g_all[:, t * E:(t + 1) * E]
            for e in range(4):
                pt = gpp.tile([128, 128], BF16)
                nc.tensor.transpose(pt[:, :], g_bf[:, 128 * e:128 * (e + 1)], id128[:, :])
                nc.scalar.activation(embT[:, e * NBS + t * 128: e * NBS + (t + 1) * 128],
                                     pt[:, :], AF.Copy)

    # ---------------- phase 2: forward LSTM scan ----------------
    # xf = emb @ Wi_f + bias is folded INTO the scan: per step, 4 extra wave
    # chunks contract embT (already resident, staged per-step into a fixed
    # [128, 32] tile via a full-width SBUF->SBUF DMA — the old [8,3072]
    # xf-row DMA wrote only 8 SBUF partitions, ~1/16 port width, ~1.5us/step)
    # and the bias enters via a K=1 ones-row matmul that also inits PSUM.
    # The emb waves don't depend on h(t-1), so they fill the PE idle while
    # the sig(o) -> h -> transpose chain of the previous step completes.
    with tc.tile_pool(name="scps", bufs=1, space="PSUM") as scps, \
         tc.tile_pool(name="wifp", bufs=1) as wifp:
        load_cast_bf16(nc, tc, wh, whf_d, H, G4)
        wif = wifp.tile([128, 4 * G4], BF16, name="wif")
        load_cast_bf16(nc, tc, wif, wif_d, E, G4)
        brow = whp.tile([1, G4], F32, name="brow")
        brow2 = whp.tile([1, G4], F32, name="brow2")
        nc.sync.dma_start(brow[0:1, :], bif_d)
        nc.sync.dma_start(brow2[0:1, :], bhf_d)
        nc.vector.tensor_add(brow[0:1, :], brow[0:1, :], brow2[0:1, :])
        biasf = whp.tile([1, G4], BF16, name="biasf")
        nc.vector.tensor_copy(biasf[:, :], brow[0:1, :])
        ones1 = whp.tile([1, BC], BF16, name="ones1")
        nc.vector.memset(ones1[:, :], 1.0)
        embT_v = embT.rearrange("p (j b s) -> p j b s", j=4, b=BC)
        # Separate PSUM tiles per gate group and parity: Tile serializes an
        # engine READ of a psum tile against PE WRITES to the same tile, so
        # each gate's activation can only overlap the later gates' matmul
        # waves if the gate groups live in different tiles.
        # Layout per quadrant (perm [g|f|r|o]): gf = cols 0:384, rt = 384:576,
        # ot = 576:768.
        # gf/rt single-buffered: their activations fire right after their own
        # wave groups (early in the step), long before the next step's
        # injection; only the o-gate tile needs parity (sig(o) is the step
        # tail, racing the next step's injection). The freed PSUM banks give
        # the h^T transpose staging a 4-deep rotation — consecutive
        # accumulation-group restarts on a just-drained PSUM bank stall the
        # PE ~1.3us (HW-measured), so spreading the per-step transposes over
        # 4 banks instead of 2 doubles the reuse gap.
        pgf = scps.tile([128, 384], F32, tag="pgf", name="pgf")
        prt = scps.tile([128, 192], F32, tag="prt", name="prt")
        pot = [scps.tile([128, 192], F32, tag=f"pot{j}", name=f"pot{j}") for j in range(2)]
        pts = [scps.tile([128, 24], BF16, tag=f"pt{j}", name=f"pt{j}") for j in range(4)]

        nc.vector.memset(c_sb[:, :], 0.0)
        nc.vector.memset(ring[:, :], 0.0)
        nc.vector.memset(pgf[:, :], 0.0)
        nc.vector.memset(prt[:, :], 0.0)
        for j in range(2):
            nc.vector.memset(pot[j][:, :], 0.0)

        # transposes chunk-major ([8,128] single-shot where the chunk
        # lies in one quarter); staging tiles rotate over 4 PSUM banks so a
        # bank's next accumulation-group restart is 4 chunk-slots away
        CHUNK_PIECES = (
            ((0, 0, 128, 0, 128),),                    # m=0: q0 cols 0:128
            ((0, 128, 64, 0, 64), (1, 0, 64, 64, 64)),  # m=1
            ((1, 64, 128, 0, 128),),                   # m=2
            ((2, 0, 128, 0, 128),),                    # m=3
            ((2, 128, 64, 0, 64), (3, 0, 64, 64, 64)),  # m=4
            ((3, 64, 128, 0, 128),),                   # m=5
        )

        def emit_transpose_chunk(u, m):
            ptm = pts[(6 * u + m) % 4]
            for (q, c0, cw, p0, pw) in CHUNK_PIECES[m]:
                nc.tensor.matmul(
                    ptm[p0:p0 + pw, 0:BC],
                    h_sb[32 * q:32 * q + BC, c0:c0 + cw],
                    id8[32 * q:32 * q + BC, :BC],
                    is_transpose=True, tile_position=(32 * q, p0),
                    skip_group_check=True)
            nc.vector.tensor_copy(ring_v[:, m, u // UB, :, u % UB],
                                  ptm[:, 0:BC])

        def emit_transposes(u):
            for m in range(6):
                emit_transpose_chunk(u, m)

        # hsfT_dram column layout: col = (s//U)*(BC*U) + b*U + (s%U)
        with tc.For_i(0, S, U, hint_engines=(mybir.EngineType.PE, mybir.EngineType.DVE)) as i:
            for u in range(U):
                ggf = pgf
                grt = prt
                got = pot[u % 2]
                prev = (u - 1) % U
                # stage embT columns for this step (and prefetch next):
                # xe[:, j*8+b] = embT[:, j*NBS + b*S + (i+u)]
                if u == 0:
                    xe0 = xflp.tile([128, 4 * BC], BF16, tag="xe", name="xe0")
                    nc.sync.dma_start(xe0[:, :], embT_v[:, :, :, ds(i, 1)])
                    xes = {0: xe0}
                if u < U - 1:
                    xen = xflp.tile([128, 4 * BC], BF16, tag="xe", name=f"xe{u + 1}")
                    (nc.gpsimd if u % 2 == 0 else nc.scalar).dma_start(
                        xen[:, :], embT_v[:, :, :, ds(i + u + 1, 1)])
                    xes[u + 1] = xen
                xe = xes.pop(u)
                # bias K=1 injection (inits PSUM), then the 4 emb wave chunks;
                # neither depends on h(u-1)
                for dst, c0, cw in ((ggf, 0, 384), (grt, 384, 192), (got, 576, 192)):
                    for q in range(4):
                        nc.tensor.matmul(
                            dst[32 * q:32 * q + BC, :],
                            lhsT=ones1[0:1, 0:BC],
                            rhs=biasf[0:1, q * 768 + c0: q * 768 + c0 + cw],
                            start=True, stop=False, tile_position=(0, 32 * q),
                            skip_group_check=True)
                for dst, c0, cw in ((ggf, 0, 384), (grt, 384, 192), (got, 576, 192)):
                    for j in range(4):
                        for q in range(4):
                            nc.tensor.matmul(
                                dst[32 * q:32 * q + BC, :],
                                lhsT=xe[:, j * BC:(j + 1) * BC],
                                rhs=wif[:, j * G4 + q * 768 + c0: j * G4 + q * 768 + c0 + cw],
                                start=False, stop=False, tile_position=(0, 32 * q),
                                skip_group_check=True)
                # k-waves gate-group major: g+f first, then r, then o. Each
                # group's activation fires as soon as its own tile's waves end
                # and overlaps the later groups' waves; only sigmoid(o) and
                # h = sig(o)*tanh(c) remain exposed after the last wave.
                # Step u-1's transpose chunk m is interleaved just before the
                # gf wave k=m of step u: only chunk 0's transpose+ring-copy is
                # chain-exposed after h(u-1); chunks 1-5 hide inside the wave
                # stream (wave k only needs ring chunk k).
                for dst, c0, cw in ((ggf, 0, 384), (grt, 384, 192), (got, 576, 192)):
                    for k in range(6):
                        if dst is ggf and u > 0:
                            emit_transpose_chunk(u - 1, k)
                        for q in range(4):
                            nc.tensor.matmul(
                                dst[32 * q:32 * q + BC, :],
                                lhsT=ring_v[:, k, prev // UB, :, prev % UB],
                                rhs=wh[:, k * G4 + q * 768 + c0: k * G4 + q * 768 + c0 + cw],
                                start=False, stop=(k == 5), tile_position=(0, 32 * q),
                                skip_group_check=True)
                    if dst is ggf:
                        if u == UB:
                            # first half of the ring is complete: stream it
                            # out now so the DMAs overlap the second half
                            for m in range(6):
                                nc.gpsimd.dma_start(
                                    hsfT_dram[m * 128:(m + 1) * 128, ds(i * BC, 128)],
                                    ring[:, m * (BC * U): m * (BC * U) + 128])
                        nc.scalar.activation(tg[0:GP, :], ggf[0:GP, 0:QH], AF.Tanh)
                        nc.scalar.activation(sig[0:GP, 0:QH], ggf[0:GP, QH:2 * QH],
                                             AF.Sigmoid)
                    elif dst is grt:
                        nc.scalar.activation(sig[0:GP, QH:2 * QH], grt[0:GP, :],
                                             AF.Sigmoid)
                nc.scalar.activation(sig[0:GP, 2 * QH:3 * QH], got[0:GP, :], AF.Sigmoid)
                # sig semantic layout: [f 0:192 | r 192:384 | o 384:576]
                nc.vector.tensor_mul(c_sb[0:GP, :], c_sb[0:GP, :], sig[0:GP, 0:QH])
                nc.vector.tensor_mul(tmp[0:GP, :], sig[0:GP, QH:2 * QH], tg[0:GP, :])
                nc.vector.tensor_add(c_sb[0:GP, :], c_sb[0:GP, :], tmp[0:GP, :])
                nc.scalar.activation(tc_[0:GP, :], c_sb[0:GP, :], AF.Tanh)
                nc.vector.tensor_mul(h_sb[0:GP, :], sig[0:GP, 2 * QH:3 * QH], tc_[0:GP, :])
            emit_transposes(U - 1)
            for m in range(6):
                nc.gpsimd.dma_start(
                    hsfT_dram[m * 128:(m + 1) * 128, ds(i * BC + 128, 128)],
                    ring[:, m * (BC * U) + 128: m * (BC * U) + 256])

    # ---------------- phase 3: qT = Wh_b^T @ hT + (bi_b + bh_b) ----------------
    with tc.tile_pool(name="whb", bufs=1) as qwp, \
         tc.tile_pool(name="qps", bufs=4, space="PSUM") as qpp, \
         tc.tile_pool(name="qtmp", bufs=1) as qtp:
        whb = qwp.tile([128, 6 * G4], BF16)
        load_cast_bf16(nc, tc, whb, whb_d, H, G4)
        bb = qtp.tile([1, G4], F32, tag="bb")
        bb2 = qtp.tile([1, G4], F32, tag="bb2")
        nc.sync.dma_start(bb[0:1, :], bib_d)
        nc.sync.dma_start(bb2[0:1, :], bhb_d)
        nc.vector.tensor_add(bb[0:1, :], bb[0:1, :], bb2[0:1, :])
        bbf = qtp.tile([1, G4], BF16, tag="bbf")
        nc.vector.tensor_copy(bbf[:, :], bb[0:1, :])
        ones8 = qtp.tile([1, BC], BF16, tag="ones8")
        nc.vector.memset(ones8[:, :], 1.0)
        for m24 in range(24):
            qp = qpp.tile([128, BC], F32)
            for k in range(6):
                nc.tensor.matmul(
                    qp[:, :],
                    lhsT=whb[:, k * G4 + m24 * 128: k * G4 + (m24 + 1) * 128],
                    rhs=ring_v[:, k, 1, :, UB - 1],
                    start=(k == 0), stop=False, skip_group_check=True)
            nc.tensor.matmul(qp[:, :], lhsT=bbf[0:1, m24 * 128:(m24 + 1) * 128],
                             rhs=ones8[0:1, :], start=False, stop=True, skip_group_check=True)
            nc.scalar.activation(qT[:, m24 * BC:(m24 + 1) * BC], qp[:, :], AF.Copy)

    # ---------------- phase 4: backward direction, fused per h-chunk ----------------
    with tc.tile_pool(name="wib", bufs=1) as wbp, \
         tc.tile_pool(name="gbps", bufs=1, space="PSUM") as gbpp, \
         tc.tile_pool(name="gbs", bufs=1) as gbsp:
        wib = wbp.tile([128, 4 * G4], BF16)
        load_cast_bf16(nc, tc, wib, wib_d, E, G4)
        HW2 = NBS // 2
        BH = BC // 2
        for m in range(6):
            for hf in range(2):
                def gb_mm(psum, gate):
                    col0 = gate * H + m * 128
                    for k in range(4):
                        for (n0, nw) in nchunks(HW2):
                            nc.tensor.matmul(
                                psum[:, n0:n0 + nw],
                                lhsT=wib[:, k * G4 + col0: k * G4 + col0 + 128],
                                rhs=embT[:, k * NBS + hf * HW2 + n0:
                                         k * NBS + hf * HW2 + n0 + nw],
                                start=(k == 0), stop=(k == 3), skip_group_check=True)

                def gb_act(dst, psum, gate, func):
                    m24 = gate * 6 + m
                    for bq in range(BH):
                        b = hf * BH + bq
                        nc.scalar.activation(
                            dst[:, bq * S:(bq + 1) * S],
                            psum[:, bq * S:(bq + 1) * S], func,
                            bias=qT[:, m24 * BC + b: m24 * BC + b + 1])

                psA = gbpp.tile([128, HW2], F32, tag="psA")
                psB = gbpp.tile([128, HW2], F32, tag="psB")
                gb_mm(psA, 0)          # r2
                gb_mm(psB, 2)          # g2
                sr = gbsp.tile([128, HW2], BF16, tag="sr")
                tg2 = gbsp.tile([128, HW2], BF16, tag="tg2")
                gb_act(sr, psA, 0, AF.Sigmoid)
                gb_act(tg2, psB, 2, AF.Tanh)
                u_sb = gbsp.tile([128, HW2], F32, tag="u")
                nc.vector.tensor_mul(u_sb[:, :], sr[:, :], tg2[:, :])
                psC = gbpp.tile([128, HW2], F32, tag="psA")
                psD = gbpp.tile([128, HW2], F32, tag="psB")
                gb_mm(psC, 1)          # f2
                gb_mm(psD, 3)          # o2
                f2s = gbsp.tile([128, HW2], F32, tag="f2s")
                o2s = gbsp.tile([128, HW2], BF16, tag="o2s")
                gb_act(f2s, psC, 1, AF.Sigmoid)
                gb_act(o2s, psD, 3, AF.Sigmoid)
                c2 = gbsp.tile([128, HW2], F32, tag="c2")
                for bq in range(BH):
                    sl = slice(bq * S, (bq + 1) * S)
                    nc.vector.tensor_tensor_scan(
                        c2[:, sl][:, ::-1], f2s[:, sl][:, ::-1],
                        u_sb[:, sl][:, ::-1], 0.0, ALU.mult, ALU.add)
                tc2 = gbsp.tile([128, HW2], BF16, tag="tc2")
                nc.scalar.activation(tc2[:, :], c2[:, :], AF.Tanh)
                hsb = gbsp.tile([128, HW2], BF16, tag="hsb")
                nc.vector.tensor_mul(hsb[:, :], o2s[:, :], tc2[:, :])
                nc.sync.dma_start(
                    hsbT_dram[m * 128:(m + 1) * 128, hf * HW2:(hf + 1) * HW2],
                    hsb[:, :])

    # ---------------- phase 5: output projection + pad bias ----------------
    with tc.tile_pool(name="prj", bufs=1) as pp, \
         tc.tile_pool(name="prjps", bufs=1, space="PSUM") as ppp, \
         tc.tile_pool(name="prjs", bufs=3) as psp:
        woutb = pp.tile([128, 12 * L], BF16)
        load_cast_bf16(nc, tc, woutb, wout_d, 2 * H, L)
        tok1 = pp.tile([1, NBS], I32)
        nc.sync.dma_start(tok1[0:1, :], tok_d.rearrange("(o n) -> o n", o=1))
        mask = pp.tile([1, NBS], F32)
        nc.vector.tensor_scalar(mask[0:1, :], tok1[0:1, :], PAD_WORD, None, ALU.is_equal)
        wmask = pp.tile([1, L], F32)
        nc.vector.memset(wmask[:, :], 0.0)
        nc.vector.memset(wmask[0:1, 0:1], PAD_BIAS)
        ones = pp.tile([1, NBS], F32)
        nc.vector.memset(ones[:, :], 1.0)
        boutf = pp.tile([1, L], F32)
        nc.sync.dma_start(boutf[0:1, :], bout_d)

        pproj = ppp.tile([L, NBS], F32)
        for kc in range(12):
            src = hsfT_dram if kc < 6 else hsbT_dram
            r0 = (kc % 6) * 128
            hs = psp.tile([128, NBS], BF16, tag="hs")
            nc.sync.dma_start(hs[:, :], src[r0:r0 + 128, :])
            if kc < 6:
                # stored col = blk*(BC*U) + b*U + u  ->  stream per-b (blk, u)
                hs_v = hs.rearrange("p (blk b u) -> p b blk u", b=BC, u=UB)
                for b in range(BC):
                    nc.tensor.matmul(pproj[:, b * S:(b + 1) * S],
                                     lhsT=woutb[:, kc * L:(kc + 1) * L],
                                     rhs=hs_v[:, b, :, :],
                                     start=(kc == 0), stop=False, skip_group_check=True)
            else:
                for (n0, nw) in nchunks(NBS):
                    nc.tensor.matmul(pproj[:, n0:n0 + nw],
                                     lhsT=woutb[:, kc * L:(kc + 1) * L],
                                     rhs=hs[:, n0:n0 + nw],
                                     start=False, stop=False, skip_group_check=True)
        for (n0, nw) in nchunks(NBS):
            nc.tensor.matmul(pproj[:, n0:n0 + nw], lhsT=wmask[0:1, :],
                             rhs=mask[0:1, n0:n0 + nw], start=False, stop=False, skip_group_check=True)
        for (n0, nw) in nchunks(NBS):
            nc.tensor.matmul(pproj[:, n0:n0 + nw], lhsT=boutf[0:1, :],
                             rhs=ones[0:1, n0:n0 + nw], start=False, stop=True, skip_group_check=True)
        outs = pp.tile([L, NBS], F32)
        nc.vector.tensor_copy(outs[:, :], pproj[:, :])
        nc.sync.dma_start(out_d, outs[:, :])

    estack.close()


def make_host_inputs(inputs, cfg, core):
    """Per-core in_map from full inputs (pure indexing / layout prep, no arithmetic)."""
    S = cfg["S"]
    perm = fwd_perm()
    toks = np.asarray(inputs["tokens"])[:S, core * BC:(core + 1) * BC]   # [S, BC]
    tokens_bm = np.ascontiguousarray(toks.T).reshape(-1).astype(np.int32)

    bf = ml_dtypes.bfloat16
    id128 = np.eye(128, dtype=bf)
    id8 = np.zeros((128, 8), dtype=bf)
    for q in range(4):
        for p in range(8):
            id8[32 * q + p, p] = 1
    f32 = lambda x: np.ascontiguousarray(np.asarray(x), dtype=np.float32)
    b16 = lambda x: np.ascontiguousarray(np.asarray(x), dtype=np.float32).astype(bf)
    uniq, inv = np.unique(tokens_bm, return_inverse=True)
    S_cfg = S
    NBS_ = BC * S_cfg
    table = np.zeros((NBS_, E), dtype=bf)
    table[:len(uniq)] = b16(np.asarray(inputs["embedding"], dtype=np.float32)[uniq])
    return {
        "tokens_bm": tokens_bm,
        "tokens_cm": inv.astype(np.int32),
        "embedding": table,
        "Wi_f_p": b16(np.asarray(inputs["Wi_f"], dtype=np.float32)[:, perm]),
        "Wh_f_p": b16(np.asarray(inputs["Wh_f"], dtype=np.float32)[:, perm]),
        "bi_f_p": f32(inputs["bi_f"])[perm].reshape(1, -1),
        "bh_f_p": f32(inputs["bh_f"])[perm].reshape(1, -1),
        "Wi_b": b16(inputs["Wi_b"]),
        "Wh_b": b16(inputs["Wh_b"]),
        "bi_b": f32(inputs["bi_b"]).reshape(1, -1),
        "bh_b": f32(inputs["bh_b"]).reshape(1, -1),
        "Wout": b16(inputs["Wout"]),
        "bout": f32(inputs["bout"]).reshape(1, -1),
        "id128": id128,
        "id8": id8,
    }


def assemble_output(results, cfg):
    S = cfg["S"]
    outs = []
    for r in results:
        o = np.asarray(r["outT"]).reshape(L, BC, S)        # [L, b, s]
        outs.append(np.transpose(o, (2, 1, 0)))            # [s, b, L]
    return np.ascontiguousarray(np.concatenate(outs, axis=1), dtype=np.float32)



def _split_excess_waits(raw: bytes, limit: int = 1) -> bytes:
    """Hoist excess per-instruction sem waits onto injected same-engine NoOps.

    walrus's setupSyncWait rejects engine/DMA instructions with too many sync
    waits; a NoOp on the same engine immediately before is semantically
    equivalent (the sequencer blocks either way) and NoOps lower fine with
    many waits (Tile's own kernel-tail drains carry 19+).
    """
    import orjson
    j = orjson.loads(raw)
    cnt = 0
    for fn in j.get("functions", []):
        for bb in fn.get("blocks") or []:
            insts = bb.get("instructions") or []
            out = []
            for inst in insts:
                si = inst.get("sync_info")
                waits = (si or {}).get("on_wait") or []
                if si and len(waits) > limit and inst.get("opcode") != "ISA":
                    excess, keep = waits[:-limit], waits[-limit:]
                    for w in excess:
                        nop = {
                            "engine": inst["engine"], "ins": [], "outs": [],
                            "name": f"waitsplit_{cnt}", "opcode": "EventSemaphore",
                            "sync_info": {"on_update": [], "on_wait": [w]},
                        }
                        if "debug" in inst:
                            nop["debug"] = inst["debug"]
                        out.append(nop)
                        cnt += 1
                    si["on_wait"] = keep
                out.append(inst)
            bb["instructions"] = out
    return orjson.dumps(j)

_BUILT = {}


def build_bass(num_devices=NCORES):
    if "full" in _BUILT:
        return _BUILT["full"]
    nc = bacc.Bacc("TRN2", target_bir_lowering=False, debug=False,
                   enable_asserts=False, num_devices=num_devices)
    with tile.TileContext(nc, pool_alloc_mode="queue") as tc:
        build_kernel(nc, tc, FULL_CFG)
    nc.compile()
    orig_to_json = nc.to_json_bytes
    nc.to_json_bytes = lambda: _split_excess_waits(orig_to_json())
    _BUILT["full"] = nc
    return nc


def run_on_hw(inputs, trace=False, trace_kwargs=None):
    from concourse import bass_utils
    nc = build_bass()
    in_maps = [make_host_inputs(inputs, FULL_CFG, core) for core in range(NCORES)]
    res = bass_utils.run_bass_kernel_spmd(
        nc, in_maps, core_ids=list(range(NCORES)),
        trace=trace, trace_kwargs=trace_kwargs or {})
    out = assemble_output(res.results, FULL_CFG)
    return out, res


def kernel(**inputs) -> np.ndarray:
    out, _ = run_on_hw(inputs, trace=False)
    return out


def run_timed(inputs, iters=3, pipeline_n=512):
    """Time NEFF executions with device-resident inputs (axon/PJRT path).

    Mirrors bass2jax.run_bass_via_pjrt's multi-core branch but device_puts the
    inputs once and loops the sharded call. The axon PJRT dispatch adds a fixed
    ~76-81ms round-trip LATENCY per synchronous call, but back-to-back
    dispatches pipeline: the device queue serializes the actual NEFF
    executions while the dispatch latency overlaps, so steady-state
    per-execution time = wall(N back-to-back executions)/N. Each round issues
    ``pipeline_n`` dispatches and blocks once at the end; we report the best
    round's per-execution average. Returns (best_seconds, all_times, out_full).
    """
    import time
    import jax
    import jax.numpy as jnp
    from jax.sharding import Mesh, PartitionSpec
    from jax.experimental.shard_map import shard_map
    from concourse import bass2jax, mybir as _mybir

    nc = build_bass()
    bass2jax.install_neuronx_cc_hook()
    in_maps = [make_host_inputs(inputs, FULL_CFG, core) for core in range(NCORES)]

    part_name = nc.partition_id_tensor.name if nc.partition_id_tensor else None
    in_names, out_names, out_avals, zero_outs = [], [], [], []
    for alloc in nc.m.functions[0].allocations:
        if not isinstance(alloc, _mybir.MemoryLocationSet):
            continue
        name = alloc.memorylocations[0].name
        if alloc.kind == "ExternalInput":
            if name != part_name:
                in_names.append(name)
        elif alloc.kind == "ExternalOutput":
            shape = tuple(alloc.tensor_shape)
            dtype = _mybir.dt.np(alloc.dtype)
            out_names.append(name)
            out_avals.append(jax.core.ShapedArray(shape, dtype))
            zero_outs.append(np.zeros(shape, dtype))
    n_params = len(in_names)
    all_in_names = in_names + out_names
    if part_name is not None:
        all_in_names = all_in_names + [part_name]

    def _body(*args):
        operands = list(args)
        if part_name is not None:
            operands.append(bass2jax.partition_id_tensor())
        outs = bass2jax._bass_exec_p.bind(
            *operands, out_avals=tuple(out_avals), in_names=tuple(all_in_names),
            out_names=tuple(out_names), lowering_input_output_aliases=(),
            sim_require_finite=True, sim_require_nnan=True, nc=nc)
        return tuple(outs)

    devices = jax.devices()[:NCORES]
    mesh = Mesh(np.asarray(devices), ("core",))
    n_outs = len(out_names)
    donate = tuple(range(n_params, n_params + n_outs))
    sharded = jax.jit(
        shard_map(_body, mesh=mesh,
                  in_specs=(PartitionSpec("core"),) * (n_params + n_outs),
                  out_specs=(PartitionSpec("core"),) * n_outs, check_rep=False),
        donate_argnums=donate, keep_unused=True)

    from jax.sharding import NamedSharding
    shard = NamedSharding(mesh, PartitionSpec("core"))
    concat_in = [
        jax.device_put(
            np.concatenate([np.asarray(in_maps[c][n]) for c in range(NCORES)], axis=0),
            shard)
        for n in in_names]
    # warmup (NEFF load + first-exec init)
    warm = [jax.device_put(
        np.zeros((NCORES * z.shape[0], *z.shape[1:]), z.dtype), shard)
        for z in zero_outs]
    out_arrs = sharded(*concat_in, *warm)
    for o in out_arrs:
        o.block_until_ready()

    times = []
    for it in range(iters):
        # donation consumes the zero output buffers: pre-make one set per
        # dispatch, all before the clock starts
        zsets = [[jax.device_put(
            np.zeros((NCORES * z.shape[0], *z.shape[1:]), z.dtype), shard)
            for z in zero_outs] for _ in range(pipeline_n)]
        for zs in zsets:
            for z in zs:
                z.block_until_ready()
        t0 = time.perf_counter()
        for zs in zsets:
            out_arrs = sharded(*concat_in, *zs)
        for o in out_arrs:
            o.block_until_ready()
        times.append((time.perf_counter() - t0) / pipeline_n)
    results = [
        {name: np.asarray(out_arrs[i]).reshape(NCORES, *out_avals[i].shape)[c]
         for i, name in enumerate(out_names)}
        for c in range(NCORES)]
    out = assemble_output(results, FULL_CFG)
    return min(times), times, out

